# revision 48
# baseline (speedup 1.0000x reference)
"""Sparse (sliding-window + sink) GQA attention block on 8 TRN2 NeuronCores.

v8: full-bf16 matmul operands; streamed x with graduated per-chunk DMA
deps; weights-first two-queue schedule with DMA issues interleaved into
the compute emission (so the 2-deep per-queue issue throttle never
blocks the ACT drain stream); PE-based v transpose; paired-head scores
matmuls (one 512-wide matmul covers two heads' windows at a j-block);
fused 256-wide attention matmuls (PSUM lazy-zero); split tail out-DMA.

Sharding: tensor-parallel over the 64 q-heads -> 8 q-heads (= 1 kv-head
group) per core; x replicated; wo partial outputs summed on host.

Per-core dataflow:
  A:  qkv projections; kv+q0 interleaved per x-tile sc-outer so ACT
      drains pipeline; x resident in SBUF (23 bf16 tiles); biases baked
      as an extra contraction row.  Warmup matmuls on a zero tile spin
      the PE clock gate up while inputs stream in.
  B:  RoPE rotate-half via a signed permutation matmul on the PE, then
      bf16 DVE mults; 0.125 score scale baked into the q-side cos/sin
      tables; v transposed on the PE against an identity rhs.
  C:  two half-passes (i<512, i>=512), software-pipelined across heads
      and interleaved under A / D.  Per head-half: scoresT into PSUM,
      exp on ACT, 0/1-mask multiply on DVE (bf16), attnT accumulated
      with fused 256-wide matmuls per j-block directly in PSUM.
      Denominators via the v ones-row; reciprocal_approx_fast; per-pair
      broadcast via a selector matmul; bf16 scale.
  D:  out[i,dd] partials (it-blocks 0-3 interleaved with C's second
      half), per-it batched bf16 DMA out alternating queues; the last
      two blocks split their DMA across both queues.
"""

import numpy as np

B, S, DIM = 1, 1024, 2880
H, HKV, HD = 64, 8, 64
GROUP = H // HKV
WINDOW = 128
THETA = 150000.0
NC = 8
HL = H // NC                 # 8 local q-heads per core
EL = HL * HD                 # 512 local q-dim
DT = (DIM + 127) // 128      # 23 d-tiles (22 full + 64)
NJ = S // 128                # 8 j/i blocks
DDC = 480                    # out-proj column chunk (6 per row-block)

_cache = {}


def _build_module():
    import concourse.bacc as bacc
    import concourse.mybir as mybir
    import concourse.tile as tile

    f32 = mybir.dt.float32
    bf16 = mybir.dt.bfloat16
    AF = mybir.ActivationFunctionType
    OP = mybir.AluOpType

    nc = bacc.Bacc("TRN2", target_bir_lowering=False, debug=False)

    def din(name, shape, dt=bf16):
        return nc.dram_tensor(name, shape, dt, kind="ExternalInput").ap()

    xt = din("xt", [128, DT * S])            # x^T tiled; tile22 row64 = 1.0
    wqg = din("wqg", [128, 4 * DT * 128])    # [(g*23+t)*128+e]; bias row64@t22
    wkvg = din("wkvg", [128, DT * 128])      # k|v; bias row64@t22
    wog = din("wog", [128, 4 * DIM])         # [2880*et + dd]
    cosq = din("cosq", [128, S])             # 0.125-scaled
    sinq = din("sinq", [128, S])             # 0.125-scaled (sign in perm)
    cosk = din("cosk", [64, S])
    sinkt = din("sinkt", [64, S])
    perm = din("perm", [128, 128])           # signed rotate-half permutation
    idm = din("idm", [128, 64])              # I64 at rows 64-127 (v transp.)
    mask01 = din("mask01", [128, 512])       # 0/1 window mask, 2 j-blocks
    sel2 = din("sel2", [128, 256])           # selA | selB
    es2 = din("es2", [128, 2], f32)          # exp(sinks), row 32*(h%4)
    out_d = nc.dram_tensor("out", [S, DIM], bf16, kind="ExternalOutput").ap()

    # x chunk boundaries (tile indices); graduated sizes, alternate queues
    XCH = [(0, 1), (1, 2), (2, 4), (4, 8), (8, 12), (12, 16), (16, 20),
           (20, 23)]

    with tile.TileContext(nc) as tc:
        import contextlib
        with contextlib.ExitStack() as ctx:
            res = ctx.enter_context(tc.tile_pool(name="res", bufs=1))
            x_ch = [res.tile([128, (b - a) * S], bf16, tag=f"xc{i}",
                             name=f"xc{i}")
                    for i, (a, b) in enumerate(XCH)]
            x_sb = []
            for i, (a, b) in enumerate(XCH):
                for t in range(a, b):
                    x_sb.append(x_ch[i][:, S * (t - a):S * (t - a + 1)])
            wq_sb = res.tile([128, 4 * DT * 128], bf16, tag="wq")
            wkv_sb = res.tile([128, DT * 128], bf16, tag="wkv")
            wo_sb = res.tile([128, 4 * DIM], bf16, tag="wo")
            cq_sb = res.tile([128, S], bf16, tag="cq")
            sq_sb = res.tile([128, S], bf16, tag="sq")
            ck_sb = res.tile([64, S], bf16, tag="ck")
            sk_sb = res.tile([64, S], bf16, tag="sk")
            pm_sb = res.tile([128, 128], bf16, tag="pm")
            id_sb = res.tile([128, 64], bf16, tag="idm")
            mk_sb = res.tile([128, 512], bf16, tag="mk")
            sel_sb = res.tile([128, 256], bf16, tag="sel")
            es_sb = res.tile([128, 2], f32, tag="es")
            ones0 = res.tile([128, 2], bf16, tag="ones0")
            kv_sb = res.tile([128, S], bf16, tag="kv")
            kr_sb = res.tile([128, S], bf16, tag="kr")
            v_sb = [res.tile([128, 65], bf16, tag=f"v{j}", name=f"v{j}")
                    for j in range(NJ)]
            qT = [res.tile([128, S], bf16, tag=f"qT{g}", name=f"qT{g}")
                  for g in range(4)]
            # qR merged per group-pair so one scores matmul spans 2 heads
            qRp = [res.tile([128, 2, S], bf16, tag=f"qRp{gp}", name=f"qRp{gp}")
                   for gp in range(2)]
            qR = [qRp[g // 2][:, g % 2, :] for g in range(4)]
            # persistent e^T tiles for j-blocks 2,3 (used by both halves)
            eT23 = {(gp, par, J): res.tile([128, 512], bf16,
                                           tag=f"e23_{gp}{par}{J}",
                                           name=f"e23_{gp}{par}{J}")
                    for gp in range(2) for par in range(2) for J in (2, 3)}
            atr = [res.tile([128, S], bf16, tag=f"atr{p}", name=f"atr{p}")
                   for p in range(4)]
            atf = [res.tile([128, S], bf16, tag=f"atf{p}", name=f"atf{p}")
                   for p in range(4)]
            # dn[2*half + hg]: heads 4*hg..4*hg+3 at partitions 0/32/64/96
            dn = [res.tile([128, 512], f32, tag=f"dn{x}", name=f"dn{x}")
                  for x in range(4)]
            rdnb = [res.tile([128, 512], bf16, tag=f"rdb{x}", name=f"rdb{x}")
                    for x in range(4)]
            rscr = res.tile([128, 512], f32, tag="rscr")
            wrm = res.tile([128, 512], bf16, tag="wrm")

            # ------- resident DMAs: weights-first, x streamed per chunk ---
            GQ = DT * 128

            def dma_xch(eng, i):
                a, b = XCH[i]
                eng.dma_start(x_ch[i][:], xt[:, S * a:S * b])

            # Only the early-needed inputs are issued up front; the rest are
            # issued mid-program, interleaved with ACT/SP work, so a DMA
            # issue never blocks the ACT compute stream at the 2-deep
            # per-queue throttle.
            nc.sync.dma_start(wkv_sb[:], wkvg[:])
            nc.scalar.dma_start(wq_sb[:, 0:GQ], wqg[:, 0:GQ])
            dma_xch(nc.sync, 0)
            dma_xch(nc.scalar, 1)
            dma_xch(nc.sync, 2)
            dma_xch(nc.scalar, 3)
            dma_xch(nc.sync, 4)
            dma_xch(nc.scalar, 5)
            dma_xch(nc.sync, 6)
            dma_xch(nc.scalar, 7)
            nc.vector.memset(wrm[:], 0.0)
            nc.vector.memset(ones0[:], 1.0)
            for x in range(4):
                nc.vector.memset(dn[x][:], 1.0)

            pp = ctx.enter_context(
                tc.tile_pool(name="pp", bufs=2, space="PSUM"))
            rp = ctx.enter_context(tc.tile_pool(name="rp", bufs=2))
            eU = ctx.enter_context(tc.tile_pool(name="eU", bufs=3))
            eP = ctx.enter_context(tc.tile_pool(name="eP", bufs=8))

            _etile = {}

            def pqt(name):
                return pp.tile([128, 512], f32, tag="pq", bufs=3, name=name)

            def pst(name):
                return pp.tile([128, 512], f32, tag="ps", bufs=2, name=name)

            def warm1():
                pw = pst("pw")
                nc.tensor.matmul(pw[:], wrm[:, 0:128], wrm[:],
                                 start=True, stop=True)

            # ------- PE warmup: spin the HAM up while inputs stream -------
            for wi in range(13):
                warm1()

            # ---------------- helpers ----------------
            def proj_kv_q0_sc(sc):
                """kv + q0 over x-tiles for one sc half; ACT drains after.

                On the sc0 (DMA-paced) pass, a warmup matmul is inserted at
                each x-chunk boundary to keep the PE p-state hot while the
                next chunk lands.
                """
                ak = pqt("pak")
                aq = pqt("paq")
                for t in range(DT):
                    dp = 128 if t < DT - 1 else DIM - 128 * (DT - 1) + 1
                    st, sp = (t == 0), (t == DT - 1)
                    nc.tensor.matmul(
                        ak[:], wkv_sb[:dp, 128 * t:128 * (t + 1)],
                        x_sb[t][:dp, 512 * sc:512 * (sc + 1)],
                        start=st, stop=sp)
                    nc.tensor.matmul(
                        aq[:], wq_sb[:dp, 128 * t:128 * (t + 1)],
                        x_sb[t][:dp, 512 * sc:512 * (sc + 1)],
                        start=st, stop=sp)
                nc.scalar.activation(kv_sb[:, 512 * sc:512 * (sc + 1)],
                                     ak[:], AF.Copy)
                nc.scalar.activation(qT[0][:, 512 * sc:512 * (sc + 1)],
                                     aq[:], AF.Copy)

            def v_transpose():
                """vT via PE transpose (identity rhs); append ones column."""
                for j in range(NJ):
                    pvt = pp.tile([128, 64], bf16, tag="pv", bufs=1,
                                  name="pv")
                    pv = pvt[:]
                    nc.tensor.matmul(
                        pv, kv_sb[64:128, 128 * j:128 * (j + 1)],
                        id_sb[64:128, 0:64],
                        start=True, stop=True, is_transpose=True)
                    nc.scalar.activation(v_sb[j][:, 0:64], pv, AF.Copy)
                    nc.vector.tensor_copy(v_sb[j][:, 64:65], ones0[:, 0:1])

            def proj_group(dst, g):
                """2x23 matmuls (ap=512) into ping-pong psums, ACT drains."""
                for sc in range(2):
                    pq = pqt("pq")
                    for t in range(DT):
                        dp = 128 if t < DT - 1 else DIM - 128 * (DT - 1) + 1
                        nc.tensor.matmul(
                            pq[:], wq_sb[:dp, (g * DT + t) * 128:
                                         (g * DT + t + 1) * 128],
                            x_sb[t][:dp, 512 * sc:512 * (sc + 1)],
                            start=(t == 0), stop=(t == DT - 1))
                    nc.scalar.activation(dst[:, 512 * sc:512 * (sc + 1)],
                                         pq[:], AF.Copy)

            def rope(dst, src, cos, sin, npart):
                """dst = src*cos + perm(src)*sin via PE perm + DVE mults."""
                for half in range(2):
                    cs = slice(512 * half, 512 * (half + 1))
                    psw = pqt("psw")
                    nc.tensor.matmul(psw[:npart, :], pm_sb[:npart, :npart],
                                     src[:npart, cs], start=True, stop=True)
                    tmp = rp.tile([128, 512], bf16, tag="tmp")
                    qc = rp.tile([128, 512], bf16, tag="qc")
                    nc.vector.tensor_tensor(tmp[:npart], psw[:npart, :],
                                            sin[:npart, cs], op=OP.mult)
                    nc.vector.tensor_tensor(qc[:npart], src[:npart, cs],
                                            cos[:npart, cs], op=OP.mult)
                    nc.vector.tensor_tensor(dst[:npart, cs], qc[:npart],
                                            tmp[:npart], op=OP.add)

            def et_tile(h, J):
                """-> (tile, col offset) of head h's 256-span at j-block J."""
                gp, par = h // 4, h % 2
                off = ((h // 2) % 2) * 256
                if J in (2, 3):
                    return eT23[(gp, par, J)], off
                return _etile[(gp, par, J)], off

            def scores_pair(gp, par, half, jp):
                """scores -> exp -> mask for 2 heads (groups 2gp, 2gp+1) of
                one row parity, 2 j-blocks.  One 512-wide matmul per
                j-block spans both heads' 256-col i-windows."""
                r0 = 64 * par
                for J in range(4 * half + 2 * jp, 4 * half + 2 * jp + 2):
                    ilo = 128 * J
                    nc2 = min(256, S - ilo)          # J=7 span is 128
                    ps = pst("ps")
                    nc.tensor.matmul(
                        ps[:, 0:2 * nc2],
                        kr_sb[r0:r0 + 64, 128 * J:128 * (J + 1)],
                        qRp[gp][r0:r0 + 64, 0:2, ilo:ilo + nc2],
                        start=True, stop=True)
                    eu = eU.tile([128, 512], bf16, tag="eu")
                    nc.scalar.activation(eu[:, :2 * nc2], ps[:, :2 * nc2],
                                         AF.Exp)
                    if J in (2, 3):
                        et = eT23[(gp, par, J)]
                    else:
                        et = eP.tile([128, 512], bf16, tag="et",
                                     name=f"et{gp}{par}_{J}")
                        _etile[(gp, par, J)] = et
                    if nc2 == 256:
                        nc.vector.tensor_tensor(et[:], eu[:],
                                                mk_sb[:], op=OP.mult)
                    else:                            # J=7: split per head
                        nc.vector.tensor_tensor(
                            et[:, 0:128], eu[:, 0:128], mk_sb[:, 0:128],
                            op=OP.mult)
                        nc.vector.tensor_tensor(
                            et[:, 256:384], eu[:, 128:256], mk_sb[:, 0:128],
                            op=OP.mult)

            def c_head_attn(h, half):
                """attnT fused 256-wide per j-block into PSUM (lazy zero)."""
                p, r0 = h // 2, 64 * (h % 2)
                dr = 32 * (h % 4)
                pb = pp.tile([65, 512], f32, tag="pb", bufs=2, name="pb")
                I0 = 4 * half
                first = True
                for J in range(max(0, I0 - 1), I0 + 4):
                    tl, off = et_tile(h, J)
                    ec0, el = off, 256
                    lo = 128 * (J - I0)
                    if J == I0 - 1:          # right half only (i-block I0)
                        ec0, el, lo = off + 128, 128, 0
                    elif J == I0 + 3:        # left half only (i-block I0+3)
                        el = 128
                    nc.tensor.matmul(
                        pb[:, lo:lo + el], v_sb[J][:, 0:65],
                        tl[:, ec0:ec0 + el],
                        start=first, stop=(J == I0 + 3),
                        skip_group_check=True)
                    first = False
                nc.vector.tensor_scalar_add(
                    dn[2 * half + h // 4][dr:dr + 1, :], pb[64:65, :],
                    es_sb[dr:dr + 1, (h // 4):(h // 4) + 1])
                nc.scalar.activation(atr[p][r0:r0 + 64,
                                            512 * half:512 * (half + 1)],
                                     pb[0:64, :], AF.Copy)

            def c_epilogue(half):
                with nc.allow_low_precision(reason="bf16 attn scale"):
                    for hg in range(2):
                        x = 2 * half + hg
                        nc.vector.reciprocal_approx_fast(rscr[:], dn[x][:])
                        nc.vector.tensor_copy(rdnb[x][:], rscr[:])
                for p in range(4):
                    prt = pst("prt")
                    nc.tensor.matmul(
                        prt[:], sel_sb[:, 128 * (p % 2):128 * (p % 2 + 1)],
                        rdnb[2 * half + p // 2][:], start=True, stop=True)
                    pc = eU.tile([128, 512], bf16, tag="prtc")
                    nc.scalar.activation(pc[:], prt[:], AF.Copy)
                    cs = slice(512 * half, 512 * (half + 1))
                    nc.vector.tensor_tensor(atf[p][:, cs], atr[p][:, cs],
                                            pc[:], op=OP.mult)

            def d_block(it):
                obt = eU.tile([128, DIM], bf16, tag="ob", bufs=3, name="obt")
                eng = nc.sync if it % 2 == 0 else nc.scalar
                alt = nc.scalar if it % 2 == 0 else nc.sync
                for dd in range(6):
                    po = pqt("po")
                    for et in range(4):
                        nc.tensor.matmul(
                            po[:, 0:DDC],
                            atf[et][:, 128 * it:128 * (it + 1)],
                            wo_sb[:, DIM * et + DDC * dd:
                                  DIM * et + DDC * (dd + 1)],
                            start=(et == 0), stop=(et == 3))
                    if dd % 2 == 0:
                        nc.scalar.activation(
                            obt[:, DDC * dd:DDC * (dd + 1)], po[:, 0:DDC],
                            AF.Copy)
                    else:
                        nc.vector.tensor_copy(
                            obt[:, DDC * dd:DDC * (dd + 1)], po[:, 0:DDC])
                    if it >= 6 and dd == 2:
                        eng.dma_start(
                            out_d[128 * it:128 * (it + 1), 0:3 * DDC],
                            obt[:, 0:3 * DDC])
                if it >= 6:
                    alt.dma_start(
                        out_d[128 * it:128 * (it + 1), 3 * DDC:DIM],
                        obt[:, 3 * DDC:DIM])
                else:
                    eng.dma_start(out_d[128 * it:128 * (it + 1), :], obt[:])

            # ---------------- Phase A + B + C-L (interleaved) -------------
            proj_kv_q0_sc(0)
            nc.scalar.dma_start(wq_sb[:, GQ:2 * GQ], wqg[:, GQ:2 * GQ])
            nc.sync.dma_start(wq_sb[:, 3 * GQ:4 * GQ], wqg[:, 3 * GQ:4 * GQ])
            proj_kv_q0_sc(1)
            nc.sync.dma_start(wq_sb[:, 2 * GQ:3 * GQ], wqg[:, 2 * GQ:3 * GQ])
            nc.sync.dma_start(pm_sb[:], perm[:])
            nc.sync.dma_start(id_sb[:], idm[:])
            nc.sync.dma_start(ck_sb[:], cosk[:])
            nc.sync.dma_start(sk_sb[:], sinkt[:])
            proj_group(qT[1], 1)
            nc.scalar.dma_start(cq_sb[:], cosq[:])
            nc.scalar.dma_start(sq_sb[:], sinq[:])
            nc.scalar.dma_start(mk_sb[:], mask01[:])
            nc.scalar.dma_start(sel_sb[:], sel2[:])
            nc.scalar.dma_start(es_sb[:], es2[:])
            nc.sync.dma_start(wo_sb[:, 0:2 * DIM], wog[:, 0:2 * DIM])
            v_transpose()
            rope(kr_sb, kv_sb, ck_sb, sk_sb, 64)
            nc.sync.dma_start(kr_sb[64:128, :], kr_sb[0:64, :])
            nc.scalar.dma_start(wo_sb[:, 2 * DIM:4 * DIM],
                                wog[:, 2 * DIM:4 * DIM])
            rope(qR[0], qT[0], cq_sb, sq_sb, 128)
            rope(qR[1], qT[1], cq_sb, sq_sb, 128)
            scores_pair(0, 0, 0, 0)
            scores_pair(0, 0, 0, 1)
            scores_pair(0, 1, 0, 0)
            scores_pair(0, 1, 0, 1)
            proj_group(qT[2], 2)
            rope(qR[2], qT[2], cq_sb, sq_sb, 128)
            c_head_attn(0, 0)
            c_head_attn(1, 0)
            proj_group(qT[3], 3)
            rope(qR[3], qT[3], cq_sb, sq_sb, 128)
            c_head_attn(2, 0)
            scores_pair(1, 0, 0, 0)
            scores_pair(1, 0, 0, 1)
            c_head_attn(3, 0)
            scores_pair(1, 1, 0, 0)
            scores_pair(1, 1, 0, 1)
            c_head_attn(4, 0)
            c_head_attn(5, 0)
            c_head_attn(6, 0)
            c_head_attn(7, 0)

            # ---------------- C-R interleaved with L-epi and D-left -------
            scores_pair(0, 0, 1, 0)
            scores_pair(0, 0, 1, 1)
            scores_pair(0, 1, 1, 0)
            scores_pair(0, 1, 1, 1)
            c_head_attn(0, 1)
            c_epilogue(0)
            c_head_attn(1, 1)
            d_block(0)
            c_head_attn(2, 1)
            c_head_attn(3, 1)
            scores_pair(1, 0, 1, 0)
            scores_pair(1, 0, 1, 1)
            d_block(1)
            c_head_attn(4, 1)
            scores_pair(1, 1, 1, 0)
            scores_pair(1, 1, 1, 1)
            c_head_attn(5, 1)
            d_block(2)
            c_head_attn(6, 1)
            c_head_attn(7, 1)
            c_epilogue(1)
            d_block(3)
            for it in range(4, NJ):
                d_block(it)

    nc.compile()
    return nc


def _host_prep(x, wq_w, wq_b, wk_w, wk_b, wv_w, wv_b, wo_w, wo_b, sinks):
    """Build per-core input maps (host-side sharding + bf16 layout prep)."""
    import ml_dtypes
    bf = ml_dtypes.bfloat16
    f = np.float32
    xT = np.ascontiguousarray(x.reshape(S, DIM).T).astype(f)   # [2880, 1024]
    xt = np.zeros((128, DT * S), f)
    for t in range(DT):
        dp = min(128, DIM - 128 * t)
        xt[:dp, S * t:S * (t + 1)] = xT[128 * t:128 * t + dp]
    xt[64, S * (DT - 1):] = 1.0                                # bias row
    xt = xt.astype(bf)

    half = HD // 2
    inv_freq = 1.0 / (THETA ** (np.arange(half, dtype=np.float64) * 2.0 / HD))
    ang = np.arange(S, dtype=np.float64)[:, None] * inv_freq   # [S, 32]
    cos_t = np.cos(ang).T.astype(f)                            # [32, S]
    sin_t = np.sin(ang).T.astype(f)
    cos64 = np.concatenate([cos_t, cos_t], 0)                  # [64, S]
    sin64 = np.concatenate([sin_t, sin_t], 0)                  # sign in perm
    scale = np.float32(HD ** -0.5)
    cosq = (np.concatenate([cos64, cos64], 0) * scale).astype(bf)
    sinq = (np.concatenate([sin64, sin64], 0) * scale).astype(bf)
    cosk = cos64.astype(bf)
    sinkt = sin64.astype(bf)

    # signed rotate-half permutation, as matmul lhsT: perm[src, a] = sign
    # out[a] = -in[a+32] for a%64<32 else +in[a-32]
    perm = np.zeros((128, 128), f)
    for a in range(128):
        if (a // 32) % 2 == 0:
            perm[a + 32, a] = -1.0
        else:
            perm[a - 32, a] = 1.0
    perm = perm.astype(bf)

    idm = np.zeros((128, 64), f)
    for i in range(64):
        idm[64 + i, i] = 1.0
    idm = idm.astype(bf)

    jj = np.arange(128)[:, None]
    ii = np.arange(512)[None, :]
    ib = ii % 256
    allow_l = (jj <= ib) & (ib < 128)
    allow_r = (ib >= 128) & (jj > ib - 128)
    mask01 = np.where(allow_l | allow_r, 1.0, 0.0).astype(bf)  # [128, 512]

    sel2 = np.zeros((128, 256), f)
    for s in range(2):                       # selA: rows 0,32; selB: 64,96
        sel2[64 * s, 128 * s:128 * s + 64] = 1.0
        sel2[64 * s + 32, 128 * s + 64:128 * (s + 1)] = 1.0
    sel2 = sel2.astype(bf)

    def tileT(w, b):
        # w [E, DIM] (+ bias b [E]) -> [128, DT*E] tiled transpose, bias@row64
        E = w.shape[0]
        o = np.zeros((128, DT * E), f)
        for t in range(DT):
            dp = min(128, DIM - 128 * t)
            o[:dp, E * t:E * (t + 1)] = w[:, 128 * t:128 * t + dp].T
        o[64, E * (DT - 1):] = b
        return o

    def esink_layout(s8):
        out = np.zeros((128, 2), f)
        for h in range(HL):
            out[32 * (h % 4), h // 4] = np.exp(np.float64(s8[h]))
        return out

    in_maps = []
    for c in range(NC):
        wq_c = wq_w[EL * c:EL * (c + 1)]                  # [512, 2880]
        wqb_c = wq_b[EL * c:EL * (c + 1)]
        wqg = np.zeros((128, 4 * DT * 128), f)
        for g in range(4):
            wqg[:, g * DT * 128:(g + 1) * DT * 128] = tileT(
                wq_c[128 * g:128 * (g + 1)], wqb_c[128 * g:128 * (g + 1)])
        wkv_c = np.concatenate([wk_w[HD * c:HD * (c + 1)],
                                wv_w[HD * c:HD * (c + 1)]], 0)
        wkvb_c = np.concatenate([wk_b[HD * c:HD * (c + 1)],
                                 wv_b[HD * c:HD * (c + 1)]])
        wo_c = np.ascontiguousarray(wo_w[:, EL * c:EL * (c + 1)].T)
        wog = np.zeros((128, 4 * DIM), f)
        for et in range(4):
            wog[:, DIM * et:DIM * (et + 1)] = wo_c[128 * et:128 * (et + 1)]
        in_maps.append({
            "xt": xt,
            "wqg": wqg.astype(bf),
            "wkvg": tileT(wkv_c, wkvb_c).astype(bf),
            "wog": wog.astype(bf),
            "cosq": cosq, "sinq": sinq, "cosk": cosk, "sinkt": sinkt,
            "perm": perm, "idm": idm, "mask01": mask01, "sel2": sel2,
            "es2": esink_layout(sinks[HL * c:HL * (c + 1)]),
        })
    return in_maps


def run_on_hw(inputs, trace=False, **kw):
    from concourse import bass_utils
    if "nc" not in _cache:
        _cache["nc"] = _build_module()
    in_maps = _host_prep(**inputs)
    res = bass_utils.run_bass_kernel_spmd(
        _cache["nc"], in_maps, core_ids=list(range(NC)), trace=trace, **kw)
    out = np.zeros((S, DIM), np.float64)
    for c in range(NC):
        out += res.results[c]["out"].astype(np.float64)
    out = (out + inputs["wo_b"].astype(np.float64)).astype(np.float32)
    return out.reshape(B, S, DIM), res


def kernel(**inputs) -> np.ndarray:
    out, _ = run_on_hw(inputs, trace=False)
    return out


# revision 51
# speedup vs baseline: 1.1419x; 1.1419x over previous
"""Sparse (sliding-window + sink) GQA attention block on 8 TRN2 NeuronCores.

v8: full-bf16 matmul operands; streamed x with graduated per-chunk DMA
deps; weights-first two-queue schedule with DMA issues interleaved into
the compute emission (so the 2-deep per-queue issue throttle never
blocks the ACT drain stream); PE-based v transpose; paired-head scores
matmuls (one 512-wide matmul covers two heads' windows at a j-block);
fused 256-wide attention matmuls (PSUM lazy-zero); split tail out-DMA.

Sharding: tensor-parallel over the 64 q-heads -> 8 q-heads (= 1 kv-head
group) per core; x replicated; wo partial outputs summed on host.

Per-core dataflow:
  A:  qkv projections; kv+q0 interleaved per x-tile sc-outer so ACT
      drains pipeline; x resident in SBUF (23 bf16 tiles); biases baked
      as an extra contraction row.  Warmup matmuls on a zero tile spin
      the PE clock gate up while inputs stream in.
  B:  RoPE rotate-half via a signed permutation matmul on the PE, then
      bf16 DVE mults; 0.125 score scale baked into the q-side cos/sin
      tables; v transposed on the PE against an identity rhs.
  C:  two half-passes (i<512, i>=512), software-pipelined across heads
      and interleaved under A / D.  Per head-half: scoresT into PSUM,
      exp on ACT, 0/1-mask multiply on DVE (bf16), attnT accumulated
      with fused 256-wide matmuls per j-block directly in PSUM.
      Denominators via the v ones-row; reciprocal_approx_fast; per-pair
      broadcast via a selector matmul; bf16 scale.
  D:  out[i,dd] partials (it-blocks 0-3 interleaved with C's second
      half), per-it batched bf16 DMA out alternating queues; the last
      two blocks split their DMA across both queues.
"""

import numpy as np

B, S, DIM = 1, 1024, 2880
H, HKV, HD = 64, 8, 64
GROUP = H // HKV
WINDOW = 128
THETA = 150000.0
NC = 8
HL = H // NC                 # 8 local q-heads per core
EL = HL * HD                 # 512 local q-dim
DT = (DIM + 127) // 128      # 23 d-tiles (22 full + 64)
NJ = S // 128                # 8 j/i blocks
DDC = 480                    # out-proj column chunk (6 per row-block)

_cache = {}


def _build_module():
    import concourse.bacc as bacc
    import concourse.mybir as mybir
    import concourse.tile as tile

    f32 = mybir.dt.float32
    bf16 = mybir.dt.bfloat16
    AF = mybir.ActivationFunctionType
    OP = mybir.AluOpType

    nc = bacc.Bacc("TRN2", target_bir_lowering=False, debug=False)

    def din(name, shape, dt=bf16):
        return nc.dram_tensor(name, shape, dt, kind="ExternalInput").ap()

    xt = din("xt", [128, DT * S])            # x^T tiled; tile22 row64 = 1.0
    wqg = din("wqg", [128, 4 * DT * 128])    # [(g*23+t)*128+e]; bias row64@t22
    wkvg = din("wkvg", [128, DT * 128])      # k|v; bias row64@t22
    wog = din("wog", [128, 4 * DIM])         # [2880*et + dd]
    cosq = din("cosq", [128, S])             # 0.125-scaled
    sinq = din("sinq", [128, S])             # 0.125-scaled (sign in perm)
    cosk = din("cosk", [64, S])
    sinkt = din("sinkt", [64, S])
    perm = din("perm", [128, 128])           # signed rotate-half permutation
    idm = din("idm", [128, 64])              # I64 at rows 64-127 (v transp.)
    mask01 = din("mask01", [128, 512])       # 0/1 window mask, 2 j-blocks
    sel2 = din("sel2", [128, 256])           # selA | selB
    es2 = din("es2", [128, 2], f32)          # exp(sinks), row 32*(h%4)
    out_d = nc.dram_tensor("out", [S, DIM], bf16, kind="ExternalOutput").ap()

    # x chunk boundaries (tile indices); graduated sizes, alternate queues
    XCH = [(0, 1), (1, 2), (2, 4), (4, 8), (8, 12), (12, 16), (16, 20),
           (20, 23)]

    with tile.TileContext(nc) as tc:
        import contextlib
        with contextlib.ExitStack() as ctx:
            res = ctx.enter_context(tc.tile_pool(name="res", bufs=1))
            x_ch = [res.tile([128, (b - a) * S], bf16, tag=f"xc{i}",
                             name=f"xc{i}")
                    for i, (a, b) in enumerate(XCH)]
            x_sb = []
            for i, (a, b) in enumerate(XCH):
                for t in range(a, b):
                    x_sb.append(x_ch[i][:, S * (t - a):S * (t - a + 1)])
            wq_sb = res.tile([128, 4 * DT * 128], bf16, tag="wq")
            wkv_sb = res.tile([128, DT * 128], bf16, tag="wkv")
            wo_sb = res.tile([128, 4 * DIM], bf16, tag="wo")
            cq_sb = res.tile([128, S], bf16, tag="cq")
            sq_sb = res.tile([128, S], bf16, tag="sq")
            ck_sb = res.tile([64, S], bf16, tag="ck")
            sk_sb = res.tile([64, S], bf16, tag="sk")
            pm_sb = res.tile([128, 128], bf16, tag="pm")
            id_sb = res.tile([128, 64], bf16, tag="idm")
            mk_sb = res.tile([128, 512], bf16, tag="mk")
            sel_sb = res.tile([128, 256], bf16, tag="sel")
            es_sb = res.tile([128, 2], f32, tag="es")
            ones0 = res.tile([128, 2], bf16, tag="ones0")
            kv_sb = res.tile([128, S], bf16, tag="kv")
            kr_sb = res.tile([128, S], bf16, tag="kr")
            v_sb = [res.tile([128, 65], bf16, tag=f"v{j}", name=f"v{j}")
                    for j in range(NJ)]
            qT = [res.tile([128, S], bf16, tag=f"qT{g}", name=f"qT{g}")
                  for g in range(4)]
            # qR merged per group-pair so one scores matmul spans 2 heads
            qRp = [res.tile([128, 2, S], bf16, tag=f"qRp{gp}", name=f"qRp{gp}")
                   for gp in range(2)]
            qR = [qRp[g // 2][:, g % 2, :] for g in range(4)]
            # persistent e^T tiles for j-blocks 2,3 (used by both halves)
            eT23 = {(gp, par, J): res.tile([128, 512], bf16,
                                           tag=f"e23_{gp}{par}{J}",
                                           name=f"e23_{gp}{par}{J}")
                    for gp in range(2) for par in range(2) for J in (2, 3)}
            atr = [res.tile([128, S], bf16, tag=f"atr{p}", name=f"atr{p}")
                   for p in range(4)]
            atf = [res.tile([128, S], bf16, tag=f"atf{p}", name=f"atf{p}")
                   for p in range(4)]
            # dn[2*half + hg]: heads 4*hg..4*hg+3 at partitions 0/32/64/96
            dn = [res.tile([128, 512], f32, tag=f"dn{x}", name=f"dn{x}")
                  for x in range(4)]
            rdnb = [res.tile([128, 512], bf16, tag=f"rdb{x}", name=f"rdb{x}")
                    for x in range(4)]
            rscr = res.tile([128, 512], f32, tag="rscr")
            wrm = res.tile([128, 512], bf16, tag="wrm")

            # ------- resident DMAs: weights-first, x streamed per chunk ---
            GQ = DT * 128

            def dma_xch(eng, i):
                a, b = XCH[i]
                eng.dma_start(x_ch[i][:], xt[:, S * a:S * b])

            # Only the early-needed inputs are issued up front; the rest are
            # issued mid-program, interleaved with ACT/SP work, so a DMA
            # issue never blocks the ACT compute stream at the 2-deep
            # per-queue throttle.
            nc.sync.dma_start(wkv_sb[:], wkvg[:])
            nc.scalar.dma_start(wq_sb[:, 0:GQ], wqg[:, 0:GQ])
            dma_xch(nc.sync, 0)
            dma_xch(nc.scalar, 1)
            dma_xch(nc.sync, 2)
            dma_xch(nc.scalar, 3)
            dma_xch(nc.sync, 4)
            dma_xch(nc.scalar, 5)
            dma_xch(nc.sync, 6)
            dma_xch(nc.scalar, 7)
            nc.vector.memset(wrm[:], 0.0)
            nc.vector.memset(ones0[:], 1.0)
            for x in range(4):
                nc.vector.memset(dn[x][:], 1.0)

            pp = ctx.enter_context(
                tc.tile_pool(name="pp", bufs=2, space="PSUM"))
            rp = ctx.enter_context(tc.tile_pool(name="rp", bufs=2))
            eU = ctx.enter_context(tc.tile_pool(name="eU", bufs=4))
            eP = ctx.enter_context(tc.tile_pool(name="eP", bufs=8))

            _etile = {}

            def pqt(name):
                return pp.tile([128, 512], f32, tag="pq", bufs=3, name=name)

            def pst(name):
                return pp.tile([128, 512], f32, tag="ps", bufs=2, name=name)

            def warm1():
                pw = pst("pw")
                nc.tensor.matmul(pw[:], wrm[:, 0:128], wrm[:],
                                 start=True, stop=True)

            # ------- PE warmup: spin the HAM up while inputs stream -------
            for wi in range(13):
                warm1()

            # ---------------- helpers ----------------
            def proj_kv_q0_sc(sc):
                """kv + q0 over x-tiles for one sc half; ACT drains after.

                On the sc0 (DMA-paced) pass, a warmup matmul is inserted at
                each x-chunk boundary to keep the PE p-state hot while the
                next chunk lands.
                """
                ak = pqt("pak")
                aq = pqt("paq")
                for t in range(DT):
                    dp = 128 if t < DT - 1 else DIM - 128 * (DT - 1) + 1
                    st, sp = (t == 0), (t == DT - 1)
                    nc.tensor.matmul(
                        ak[:], wkv_sb[:dp, 128 * t:128 * (t + 1)],
                        x_sb[t][:dp, 512 * sc:512 * (sc + 1)],
                        start=st, stop=sp)
                    nc.tensor.matmul(
                        aq[:], wq_sb[:dp, 128 * t:128 * (t + 1)],
                        x_sb[t][:dp, 512 * sc:512 * (sc + 1)],
                        start=st, stop=sp)
                nc.scalar.activation(kv_sb[:, 512 * sc:512 * (sc + 1)],
                                     ak[:], AF.Copy)
                nc.scalar.activation(qT[0][:, 512 * sc:512 * (sc + 1)],
                                     aq[:], AF.Copy)

            def v_transpose():
                """vT via PE transpose (identity rhs); append ones column."""
                for j in range(NJ):
                    pvt = pp.tile([128, 64], bf16, tag="pv", bufs=1,
                                  name="pv")
                    pv = pvt[:]
                    nc.tensor.matmul(
                        pv, kv_sb[64:128, 128 * j:128 * (j + 1)],
                        id_sb[64:128, 0:64],
                        start=True, stop=True, is_transpose=True)
                    nc.scalar.activation(v_sb[j][:, 0:64], pv, AF.Copy)
                    nc.vector.tensor_copy(v_sb[j][:, 64:65], ones0[:, 0:1])

            def proj_group(dst, g):
                """2x23 matmuls (ap=512) into ping-pong psums, ACT drains."""
                for sc in range(2):
                    pq = pqt("pq")
                    for t in range(DT):
                        dp = 128 if t < DT - 1 else DIM - 128 * (DT - 1) + 1
                        nc.tensor.matmul(
                            pq[:], wq_sb[:dp, (g * DT + t) * 128:
                                         (g * DT + t + 1) * 128],
                            x_sb[t][:dp, 512 * sc:512 * (sc + 1)],
                            start=(t == 0), stop=(t == DT - 1))
                    nc.scalar.activation(dst[:, 512 * sc:512 * (sc + 1)],
                                         pq[:], AF.Copy)

            def rope(dst, src, cos, sin, npart):
                """dst = src*cos + perm(src)*sin via PE perm + DVE mults."""
                for half in range(2):
                    cs = slice(512 * half, 512 * (half + 1))
                    psw = pqt("psw")
                    nc.tensor.matmul(psw[:npart, :], pm_sb[:npart, :npart],
                                     src[:npart, cs], start=True, stop=True)
                    tmp = rp.tile([128, 512], bf16, tag="tmp")
                    qc = rp.tile([128, 512], bf16, tag="qc")
                    nc.vector.tensor_tensor(tmp[:npart], psw[:npart, :],
                                            sin[:npart, cs], op=OP.mult)
                    nc.vector.tensor_tensor(qc[:npart], src[:npart, cs],
                                            cos[:npart, cs], op=OP.mult)
                    nc.vector.tensor_tensor(dst[:npart, cs], qc[:npart],
                                            tmp[:npart], op=OP.add)

            def et_tile(h, J):
                """-> (tile, col offset) of head h's 256-span at j-block J."""
                gp, par = h // 4, h % 2
                off = ((h // 2) % 2) * 256
                if J in (2, 3):
                    return eT23[(gp, par, J)], off
                return _etile[(gp, par, J)], off

            def scores_pair(gp, par, half, jp):
                """scores -> exp -> mask for 2 heads (groups 2gp, 2gp+1) of
                one row parity, 2 j-blocks.  One 512-wide matmul per
                j-block spans both heads' 256-col i-windows."""
                r0 = 64 * par
                for J in range(4 * half + 2 * jp, 4 * half + 2 * jp + 2):
                    ilo = 128 * J
                    nc2 = min(256, S - ilo)          # J=7 span is 128
                    ps = pst("ps")
                    nc.tensor.matmul(
                        ps[:, 0:2 * nc2],
                        kr_sb[r0:r0 + 64, 128 * J:128 * (J + 1)],
                        qRp[gp][r0:r0 + 64, 0:2, ilo:ilo + nc2],
                        start=True, stop=True)
                    eu = eU.tile([128, 512], bf16, tag="eu")
                    nc.scalar.activation(eu[:, :2 * nc2], ps[:, :2 * nc2],
                                         AF.Exp)
                    if J in (2, 3):
                        et = eT23[(gp, par, J)]
                    else:
                        et = eP.tile([128, 512], bf16, tag="et",
                                     name=f"et{gp}{par}_{J}")
                        _etile[(gp, par, J)] = et
                    if nc2 == 256:
                        nc.gpsimd.tensor_tensor(et[:], eu[:],
                                                mk_sb[:], op=OP.mult)
                    else:                            # J=7: split per head
                        nc.gpsimd.tensor_tensor(
                            et[:, 0:128], eu[:, 0:128], mk_sb[:, 0:128],
                            op=OP.mult)
                        nc.gpsimd.tensor_tensor(
                            et[:, 256:384], eu[:, 128:256], mk_sb[:, 0:128],
                            op=OP.mult)

            def c_head_attn(h, half):
                """attnT fused 256-wide per j-block into PSUM (lazy zero)."""
                p, r0 = h // 2, 64 * (h % 2)
                dr = 32 * (h % 4)
                pb = pp.tile([65, 512], f32, tag="pb", bufs=2, name="pb")
                I0 = 4 * half
                first = True
                for J in range(max(0, I0 - 1), I0 + 4):
                    tl, off = et_tile(h, J)
                    ec0, el = off, 256
                    lo = 128 * (J - I0)
                    if J == I0 - 1:          # right half only (i-block I0)
                        ec0, el, lo = off + 128, 128, 0
                    elif J == I0 + 3:        # left half only (i-block I0+3)
                        el = 128
                    nc.tensor.matmul(
                        pb[:, lo:lo + el], v_sb[J][:, 0:65],
                        tl[:, ec0:ec0 + el],
                        start=first, stop=(J == I0 + 3),
                        skip_group_check=True)
                    first = False
                nc.vector.tensor_scalar_add(
                    dn[2 * half + h // 4][dr:dr + 1, :], pb[64:65, :],
                    es_sb[dr:dr + 1, (h // 4):(h // 4) + 1])
                nc.scalar.activation(atr[p][r0:r0 + 64,
                                            512 * half:512 * (half + 1)],
                                     pb[0:64, :], AF.Copy)

            def c_epilogue(half):
                with nc.allow_low_precision(reason="bf16 attn scale"):
                    for hg in range(2):
                        x = 2 * half + hg
                        nc.vector.reciprocal_approx_fast(rscr[:], dn[x][:])
                        nc.vector.tensor_copy(rdnb[x][:], rscr[:])
                for p in range(4):
                    prt = pst("prt")
                    nc.tensor.matmul(
                        prt[:], sel_sb[:, 128 * (p % 2):128 * (p % 2 + 1)],
                        rdnb[2 * half + p // 2][:], start=True, stop=True)
                    pc = eU.tile([128, 512], bf16, tag="prtc")
                    nc.scalar.activation(pc[:], prt[:], AF.Copy)
                    cs = slice(512 * half, 512 * (half + 1))
                    nc.gpsimd.tensor_tensor(atf[p][:, cs], atr[p][:, cs],
                                            pc[:], op=OP.mult)

            def d_block(it):
                obt = eU.tile([128, DIM], bf16, tag="ob", bufs=3, name="obt")
                eng = nc.sync if it % 2 == 0 else nc.scalar
                alt = nc.scalar if it % 2 == 0 else nc.sync
                for dd in range(6):
                    po = pqt("po")
                    for et in range(4):
                        nc.tensor.matmul(
                            po[:, 0:DDC],
                            atf[et][:, 128 * it:128 * (it + 1)],
                            wo_sb[:, DIM * et + DDC * dd:
                                  DIM * et + DDC * (dd + 1)],
                            start=(et == 0), stop=(et == 3))
                    if dd % 2 == 0:
                        nc.scalar.activation(
                            obt[:, DDC * dd:DDC * (dd + 1)], po[:, 0:DDC],
                            AF.Copy)
                    else:
                        nc.vector.tensor_copy(
                            obt[:, DDC * dd:DDC * (dd + 1)], po[:, 0:DDC])
                    if it >= 6 and dd == 2:
                        eng.dma_start(
                            out_d[128 * it:128 * (it + 1), 0:3 * DDC],
                            obt[:, 0:3 * DDC])
                if it >= 6:
                    alt.dma_start(
                        out_d[128 * it:128 * (it + 1), 3 * DDC:DIM],
                        obt[:, 3 * DDC:DIM])
                else:
                    eng.dma_start(out_d[128 * it:128 * (it + 1), :], obt[:])

            # ---------------- Phase A + B + C-L (interleaved) -------------
            proj_kv_q0_sc(0)
            nc.scalar.dma_start(wq_sb[:, GQ:2 * GQ], wqg[:, GQ:2 * GQ])
            nc.sync.dma_start(wq_sb[:, 3 * GQ:4 * GQ], wqg[:, 3 * GQ:4 * GQ])
            proj_kv_q0_sc(1)
            nc.sync.dma_start(wq_sb[:, 2 * GQ:3 * GQ], wqg[:, 2 * GQ:3 * GQ])
            nc.sync.dma_start(pm_sb[:], perm[:])
            nc.sync.dma_start(id_sb[:], idm[:])
            nc.sync.dma_start(ck_sb[:], cosk[:])
            nc.sync.dma_start(sk_sb[:], sinkt[:])
            proj_group(qT[1], 1)
            nc.scalar.dma_start(cq_sb[:], cosq[:])
            nc.scalar.dma_start(sq_sb[:], sinq[:])
            nc.scalar.dma_start(mk_sb[:], mask01[:])
            nc.scalar.dma_start(sel_sb[:], sel2[:])
            nc.scalar.dma_start(es_sb[:], es2[:])
            nc.sync.dma_start(wo_sb[:, 0:2 * DIM], wog[:, 0:2 * DIM])
            v_transpose()
            rope(kr_sb, kv_sb, ck_sb, sk_sb, 64)
            nc.sync.dma_start(kr_sb[64:128, :], kr_sb[0:64, :])
            nc.scalar.dma_start(wo_sb[:, 2 * DIM:4 * DIM],
                                wog[:, 2 * DIM:4 * DIM])
            rope(qR[0], qT[0], cq_sb, sq_sb, 128)
            rope(qR[1], qT[1], cq_sb, sq_sb, 128)
            scores_pair(0, 0, 0, 0)
            scores_pair(0, 0, 0, 1)
            scores_pair(0, 1, 0, 0)
            scores_pair(0, 1, 0, 1)
            proj_group(qT[2], 2)
            rope(qR[2], qT[2], cq_sb, sq_sb, 128)
            c_head_attn(0, 0)
            c_head_attn(1, 0)
            proj_group(qT[3], 3)
            rope(qR[3], qT[3], cq_sb, sq_sb, 128)
            c_head_attn(2, 0)
            scores_pair(1, 0, 0, 0)
            scores_pair(1, 0, 0, 1)
            c_head_attn(3, 0)
            scores_pair(1, 1, 0, 0)
            scores_pair(1, 1, 0, 1)
            c_head_attn(4, 0)
            c_head_attn(5, 0)
            c_head_attn(6, 0)
            c_head_attn(7, 0)

            # ---------------- C-R interleaved with L-epi and D-left -------
            scores_pair(0, 0, 1, 0)
            scores_pair(0, 0, 1, 1)
            scores_pair(0, 1, 1, 0)
            scores_pair(0, 1, 1, 1)
            c_head_attn(0, 1)
            c_epilogue(0)
            c_head_attn(1, 1)
            d_block(0)
            c_head_attn(2, 1)
            c_head_attn(3, 1)
            scores_pair(1, 0, 1, 0)
            scores_pair(1, 0, 1, 1)
            d_block(1)
            c_head_attn(4, 1)
            scores_pair(1, 1, 1, 0)
            scores_pair(1, 1, 1, 1)
            c_head_attn(5, 1)
            d_block(2)
            c_head_attn(6, 1)
            c_head_attn(7, 1)
            c_epilogue(1)
            d_block(3)
            for it in range(4, NJ):
                d_block(it)

    nc.compile()
    return nc


def _host_prep(x, wq_w, wq_b, wk_w, wk_b, wv_w, wv_b, wo_w, wo_b, sinks):
    """Build per-core input maps (host-side sharding + bf16 layout prep)."""
    import ml_dtypes
    bf = ml_dtypes.bfloat16
    f = np.float32
    xT = np.ascontiguousarray(x.reshape(S, DIM).T).astype(f)   # [2880, 1024]
    xt = np.zeros((128, DT * S), f)
    for t in range(DT):
        dp = min(128, DIM - 128 * t)
        xt[:dp, S * t:S * (t + 1)] = xT[128 * t:128 * t + dp]
    xt[64, S * (DT - 1):] = 1.0                                # bias row
    xt = xt.astype(bf)

    half = HD // 2
    inv_freq = 1.0 / (THETA ** (np.arange(half, dtype=np.float64) * 2.0 / HD))
    ang = np.arange(S, dtype=np.float64)[:, None] * inv_freq   # [S, 32]
    cos_t = np.cos(ang).T.astype(f)                            # [32, S]
    sin_t = np.sin(ang).T.astype(f)
    cos64 = np.concatenate([cos_t, cos_t], 0)                  # [64, S]
    sin64 = np.concatenate([sin_t, sin_t], 0)                  # sign in perm
    scale = np.float32(HD ** -0.5)
    cosq = (np.concatenate([cos64, cos64], 0) * scale).astype(bf)
    sinq = (np.concatenate([sin64, sin64], 0) * scale).astype(bf)
    cosk = cos64.astype(bf)
    sinkt = sin64.astype(bf)

    # signed rotate-half permutation, as matmul lhsT: perm[src, a] = sign
    # out[a] = -in[a+32] for a%64<32 else +in[a-32]
    perm = np.zeros((128, 128), f)
    for a in range(128):
        if (a // 32) % 2 == 0:
            perm[a + 32, a] = -1.0
        else:
            perm[a - 32, a] = 1.0
    perm = perm.astype(bf)

    idm = np.zeros((128, 64), f)
    for i in range(64):
        idm[64 + i, i] = 1.0
    idm = idm.astype(bf)

    jj = np.arange(128)[:, None]
    ii = np.arange(512)[None, :]
    ib = ii % 256
    allow_l = (jj <= ib) & (ib < 128)
    allow_r = (ib >= 128) & (jj > ib - 128)
    mask01 = np.where(allow_l | allow_r, 1.0, 0.0).astype(bf)  # [128, 512]

    sel2 = np.zeros((128, 256), f)
    for s in range(2):                       # selA: rows 0,32; selB: 64,96
        sel2[64 * s, 128 * s:128 * s + 64] = 1.0
        sel2[64 * s + 32, 128 * s + 64:128 * (s + 1)] = 1.0
    sel2 = sel2.astype(bf)

    def tileT(w, b):
        # w [E, DIM] (+ bias b [E]) -> [128, DT*E] tiled transpose, bias@row64
        E = w.shape[0]
        o = np.zeros((128, DT * E), f)
        for t in range(DT):
            dp = min(128, DIM - 128 * t)
            o[:dp, E * t:E * (t + 1)] = w[:, 128 * t:128 * t + dp].T
        o[64, E * (DT - 1):] = b
        return o

    def esink_layout(s8):
        out = np.zeros((128, 2), f)
        for h in range(HL):
            out[32 * (h % 4), h // 4] = np.exp(np.float64(s8[h]))
        return out

    in_maps = []
    for c in range(NC):
        wq_c = wq_w[EL * c:EL * (c + 1)]                  # [512, 2880]
        wqb_c = wq_b[EL * c:EL * (c + 1)]
        wqg = np.zeros((128, 4 * DT * 128), f)
        for g in range(4):
            wqg[:, g * DT * 128:(g + 1) * DT * 128] = tileT(
                wq_c[128 * g:128 * (g + 1)], wqb_c[128 * g:128 * (g + 1)])
        wkv_c = np.concatenate([wk_w[HD * c:HD * (c + 1)],
                                wv_w[HD * c:HD * (c + 1)]], 0)
        wkvb_c = np.concatenate([wk_b[HD * c:HD * (c + 1)],
                                 wv_b[HD * c:HD * (c + 1)]])
        wo_c = np.ascontiguousarray(wo_w[:, EL * c:EL * (c + 1)].T)
        wog = np.zeros((128, 4 * DIM), f)
        for et in range(4):
            wog[:, DIM * et:DIM * (et + 1)] = wo_c[128 * et:128 * (et + 1)]
        in_maps.append({
            "xt": xt,
            "wqg": wqg.astype(bf),
            "wkvg": tileT(wkv_c, wkvb_c).astype(bf),
            "wog": wog.astype(bf),
            "cosq": cosq, "sinq": sinq, "cosk": cosk, "sinkt": sinkt,
            "perm": perm, "idm": idm, "mask01": mask01, "sel2": sel2,
            "es2": esink_layout(sinks[HL * c:HL * (c + 1)]),
        })
    return in_maps


def run_on_hw(inputs, trace=False, **kw):
    from concourse import bass_utils
    if "nc" not in _cache:
        _cache["nc"] = _build_module()
    in_maps = _host_prep(**inputs)
    res = bass_utils.run_bass_kernel_spmd(
        _cache["nc"], in_maps, core_ids=list(range(NC)), trace=trace, **kw)
    out = np.zeros((S, DIM), np.float64)
    for c in range(NC):
        out += res.results[c]["out"].astype(np.float64)
    out = (out + inputs["wo_b"].astype(np.float64)).astype(np.float32)
    return out.reshape(B, S, DIM), res


def kernel(**inputs) -> np.ndarray:
    out, _ = run_on_hw(inputs, trace=False)
    return out


# revision 55
# speedup vs baseline: 1.1532x; 1.0099x over previous
"""Sparse (sliding-window + sink) GQA attention block on 8 TRN2 NeuronCores.

v8: full-bf16 matmul operands; streamed x with graduated per-chunk DMA
deps; weights-first two-queue schedule with DMA issues interleaved into
the compute emission (so the 2-deep per-queue issue throttle never
blocks the ACT drain stream); PE-based v transpose; paired-head scores
matmuls (one 512-wide matmul covers two heads' windows at a j-block);
fused 256-wide attention matmuls (PSUM lazy-zero); split tail out-DMA.

Sharding: tensor-parallel over the 64 q-heads -> 8 q-heads (= 1 kv-head
group) per core; x replicated; wo partial outputs summed on host.

Per-core dataflow:
  A:  qkv projections; kv+q0 interleaved per x-tile sc-outer so ACT
      drains pipeline; x resident in SBUF (23 bf16 tiles); biases baked
      as an extra contraction row.  Warmup matmuls on a zero tile spin
      the PE clock gate up while inputs stream in.
  B:  RoPE rotate-half via a signed permutation matmul on the PE, then
      bf16 DVE mults; 0.125 score scale baked into the q-side cos/sin
      tables; v transposed on the PE against an identity rhs.
  C:  two half-passes (i<512, i>=512), software-pipelined across heads
      and interleaved under A / D.  Per head-half: scoresT into PSUM,
      exp on ACT, 0/1-mask multiply on DVE (bf16), attnT accumulated
      with fused 256-wide matmuls per j-block directly in PSUM.
      Denominators via the v ones-row; reciprocal_approx_fast; per-pair
      broadcast via a selector matmul; bf16 scale.
  D:  out[i,dd] partials (it-blocks 0-3 interleaved with C's second
      half), per-it batched bf16 DMA out alternating queues; the last
      two blocks split their DMA across both queues.
"""

import numpy as np

B, S, DIM = 1, 1024, 2880
H, HKV, HD = 64, 8, 64
GROUP = H // HKV
WINDOW = 128
THETA = 150000.0
NC = 8
HL = H // NC                 # 8 local q-heads per core
EL = HL * HD                 # 512 local q-dim
DT = (DIM + 127) // 128      # 23 d-tiles (22 full + 64)
NJ = S // 128                # 8 j/i blocks
DDC = 480                    # out-proj column chunk (6 per row-block)

_cache = {}


def _build_module():
    import concourse.bacc as bacc
    import concourse.mybir as mybir
    import concourse.tile as tile

    f32 = mybir.dt.float32
    bf16 = mybir.dt.bfloat16
    AF = mybir.ActivationFunctionType
    OP = mybir.AluOpType

    nc = bacc.Bacc("TRN2", target_bir_lowering=False, debug=False)

    def din(name, shape, dt=bf16):
        return nc.dram_tensor(name, shape, dt, kind="ExternalInput").ap()

    xt = din("xt", [128, DT * S])            # x^T tiled; tile22 row64 = 1.0
    wqg = din("wqg", [128, 4 * DT * 128])    # [(g*23+t)*128+e]; bias row64@t22
    wkvg = din("wkvg", [128, DT * 128])      # k|v; bias row64@t22
    wog = din("wog", [128, 4 * DIM])         # [2880*et + dd]
    cosq = din("cosq", [128, S])             # 0.125-scaled
    sinq = din("sinq", [128, S])             # 0.125-scaled (sign in perm)
    cosk = din("cosk", [64, S])
    sinkt = din("sinkt", [64, S])
    perm = din("perm", [128, 128])           # signed rotate-half permutation
    idm = din("idm", [128, 64])              # I64 at rows 64-127 (v transp.)
    mask01 = din("mask01", [128, 512])       # 0/1 window mask, 2 j-blocks
    sel2 = din("sel2", [128, 256])           # selA | selB
    es2 = din("es2", [128, 2], f32)          # exp(sinks), row 32*(h%4)
    out_d = nc.dram_tensor("out", [S, DIM], bf16, kind="ExternalOutput").ap()

    # x chunk boundaries (tile indices); graduated sizes, alternate queues
    XCH = [(0, 1), (1, 2), (2, 4), (4, 8), (8, 12), (12, 16), (16, 20),
           (20, 23)]

    with tile.TileContext(nc) as tc:
        import contextlib
        with contextlib.ExitStack() as ctx:
            res = ctx.enter_context(tc.tile_pool(name="res", bufs=1))
            x_ch = [res.tile([128, (b - a) * S], bf16, tag=f"xc{i}",
                             name=f"xc{i}")
                    for i, (a, b) in enumerate(XCH)]
            x_sb = []
            for i, (a, b) in enumerate(XCH):
                for t in range(a, b):
                    x_sb.append(x_ch[i][:, S * (t - a):S * (t - a + 1)])
            wq_sb = res.tile([128, 4 * DT * 128], bf16, tag="wq")
            wkv_sb = res.tile([128, DT * 128], bf16, tag="wkv")
            wo_sb = res.tile([128, 4 * DIM], bf16, tag="wo")
            cq_sb = res.tile([128, S], bf16, tag="cq")
            sq_sb = res.tile([128, S], bf16, tag="sq")
            ck_sb = res.tile([64, S], bf16, tag="ck")
            sk_sb = res.tile([64, S], bf16, tag="sk")
            pm_sb = res.tile([128, 128], bf16, tag="pm")
            id_sb = res.tile([128, 64], bf16, tag="idm")
            mk_sb = res.tile([128, 512], bf16, tag="mk")
            sel_sb = res.tile([128, 256], bf16, tag="sel")
            es_sb = res.tile([128, 2], f32, tag="es")
            ones0 = res.tile([128, 2], bf16, tag="ones0")
            kv_sb = res.tile([128, S], bf16, tag="kv")
            kr_sb = res.tile([128, S], bf16, tag="kr")
            v_sb = [res.tile([128, 65], bf16, tag=f"v{j}", name=f"v{j}")
                    for j in range(NJ)]
            qT = [res.tile([128, S], bf16, tag=f"qT{g}", name=f"qT{g}")
                  for g in range(4)]
            # qR merged per group-pair so one scores matmul spans 2 heads
            qRp = [res.tile([128, 2, S], bf16, tag=f"qRp{gp}", name=f"qRp{gp}")
                   for gp in range(2)]
            qR = [qRp[g // 2][:, g % 2, :] for g in range(4)]
            # persistent e^T tiles for j-blocks 2,3 (used by both halves)
            eT23 = {(gp, par, J): res.tile([128, 512], bf16,
                                           tag=f"e23_{gp}{par}{J}",
                                           name=f"e23_{gp}{par}{J}")
                    for gp in range(2) for par in range(2) for J in (2, 3)}
            atr = [res.tile([128, S], bf16, tag=f"atr{p}", name=f"atr{p}")
                   for p in range(4)]
            atf = [res.tile([128, S], bf16, tag=f"atf{p}", name=f"atf{p}")
                   for p in range(4)]
            # dn[2*half + hg]: heads 4*hg..4*hg+3 at partitions 0/32/64/96
            dn = [res.tile([128, 512], f32, tag=f"dn{x}", name=f"dn{x}")
                  for x in range(4)]
            rdnb = [res.tile([128, 512], bf16, tag=f"rdb{x}", name=f"rdb{x}")
                    for x in range(4)]
            rscr = res.tile([128, 512], f32, tag="rscr")
            wrm = res.tile([128, 512], bf16, tag="wrm")

            # ------- resident DMAs: weights-first, x streamed per chunk ---
            GQ = DT * 128

            def dma_xch(eng, i):
                a, b = XCH[i]
                eng.dma_start(x_ch[i][:], xt[:, S * a:S * b])

            # Only the early-needed inputs are issued up front; the rest are
            # issued mid-program, interleaved with ACT/SP work, so a DMA
            # issue never blocks the ACT compute stream at the 2-deep
            # per-queue throttle.
            nc.sync.dma_start(wkv_sb[:], wkvg[:])
            nc.scalar.dma_start(wq_sb[:, 0:GQ], wqg[:, 0:GQ])
            dma_xch(nc.sync, 0)
            dma_xch(nc.scalar, 1)
            dma_xch(nc.sync, 2)
            dma_xch(nc.scalar, 3)
            dma_xch(nc.sync, 4)
            dma_xch(nc.scalar, 5)
            dma_xch(nc.sync, 6)
            dma_xch(nc.scalar, 7)
            nc.vector.memset(wrm[:], 0.0)
            nc.vector.memset(ones0[:], 1.0)
            for x in range(4):
                nc.vector.memset(dn[x][:], 1.0)

            pp = ctx.enter_context(
                tc.tile_pool(name="pp", bufs=2, space="PSUM"))
            rp = ctx.enter_context(tc.tile_pool(name="rp", bufs=2))
            eU = ctx.enter_context(tc.tile_pool(name="eU", bufs=3))
            eP = ctx.enter_context(tc.tile_pool(name="eP", bufs=8))

            _etile = {}

            def pqt(name):
                return pp.tile([128, 512], f32, tag="pq", bufs=3, name=name)

            def pst(name):
                return pp.tile([128, 512], f32, tag="ps", bufs=2, name=name)

            def warm1():
                pw = pst("pw")
                nc.tensor.matmul(pw[:], wrm[:, 0:128], wrm[:],
                                 start=True, stop=True)

            # ------- PE warmup: spin the HAM up while inputs stream -------
            for wi in range(15):
                warm1()

            # ---------------- helpers ----------------
            def proj_kv_q0_sc(sc):
                """kv + q0 over x-tiles for one sc half; ACT drains after.

                On the sc0 (DMA-paced) pass, a warmup matmul is inserted at
                each x-chunk boundary to keep the PE p-state hot while the
                next chunk lands.
                """
                ak = pqt("pak")
                aq = pqt("paq")
                for t in range(DT):
                    dp = 128 if t < DT - 1 else DIM - 128 * (DT - 1) + 1
                    st, sp = (t == 0), (t == DT - 1)
                    nc.tensor.matmul(
                        ak[:], wkv_sb[:dp, 128 * t:128 * (t + 1)],
                        x_sb[t][:dp, 512 * sc:512 * (sc + 1)],
                        start=st, stop=sp)
                    nc.tensor.matmul(
                        aq[:], wq_sb[:dp, 128 * t:128 * (t + 1)],
                        x_sb[t][:dp, 512 * sc:512 * (sc + 1)],
                        start=st, stop=sp)
                nc.scalar.activation(kv_sb[:, 512 * sc:512 * (sc + 1)],
                                     ak[:], AF.Copy)
                nc.scalar.activation(qT[0][:, 512 * sc:512 * (sc + 1)],
                                     aq[:], AF.Copy)

            def v_transpose():
                """vT via PE transpose (identity rhs); append ones column."""
                for j in range(NJ):
                    pvt = pp.tile([128, 64], bf16, tag="pv", bufs=1,
                                  name="pv")
                    pv = pvt[:]
                    nc.tensor.matmul(
                        pv, kv_sb[64:128, 128 * j:128 * (j + 1)],
                        id_sb[64:128, 0:64],
                        start=True, stop=True, is_transpose=True)
                    nc.scalar.activation(v_sb[j][:, 0:64], pv, AF.Copy)
                    nc.vector.tensor_copy(v_sb[j][:, 64:65], ones0[:, 0:1])

            def proj_group(dst, g):
                """2x23 matmuls (ap=512) into ping-pong psums, ACT drains."""
                for sc in range(2):
                    pq = pqt("pq")
                    for t in range(DT):
                        dp = 128 if t < DT - 1 else DIM - 128 * (DT - 1) + 1
                        nc.tensor.matmul(
                            pq[:], wq_sb[:dp, (g * DT + t) * 128:
                                         (g * DT + t + 1) * 128],
                            x_sb[t][:dp, 512 * sc:512 * (sc + 1)],
                            start=(t == 0), stop=(t == DT - 1))
                    nc.scalar.activation(dst[:, 512 * sc:512 * (sc + 1)],
                                         pq[:], AF.Copy)

            def rope(dst, src, cos, sin, npart):
                """dst = src*cos + perm(src)*sin via PE perm + DVE mults."""
                for half in range(2):
                    cs = slice(512 * half, 512 * (half + 1))
                    psw = pqt("psw")
                    nc.tensor.matmul(psw[:npart, :], pm_sb[:npart, :npart],
                                     src[:npart, cs], start=True, stop=True)
                    tmp = rp.tile([128, 512], bf16, tag="tmp")
                    qc = rp.tile([128, 512], bf16, tag="qc")
                    nc.vector.tensor_tensor(tmp[:npart], psw[:npart, :],
                                            sin[:npart, cs], op=OP.mult)
                    nc.vector.tensor_tensor(qc[:npart], src[:npart, cs],
                                            cos[:npart, cs], op=OP.mult)
                    nc.vector.tensor_tensor(dst[:npart, cs], qc[:npart],
                                            tmp[:npart], op=OP.add)

            def et_tile(h, J):
                """-> (tile, col offset) of head h's 256-span at j-block J."""
                gp, par = h // 4, h % 2
                off = ((h // 2) % 2) * 256
                if J in (2, 3):
                    return eT23[(gp, par, J)], off
                return _etile[(gp, par, J)], off

            def scores_pair(gp, par, half, jp):
                """scores -> exp -> mask for 2 heads (groups 2gp, 2gp+1) of
                one row parity, 2 j-blocks.  One 512-wide matmul per
                j-block spans both heads' 256-col i-windows."""
                r0 = 64 * par
                for J in range(4 * half + 2 * jp, 4 * half + 2 * jp + 2):
                    ilo = 128 * J
                    nc2 = min(256, S - ilo)          # J=7 span is 128
                    ps = pst("ps")
                    nc.tensor.matmul(
                        ps[:, 0:2 * nc2],
                        kr_sb[r0:r0 + 64, 128 * J:128 * (J + 1)],
                        qRp[gp][r0:r0 + 64, 0:2, ilo:ilo + nc2],
                        start=True, stop=True)
                    eu = eU.tile([128, 512], bf16, tag="eu")
                    nc.scalar.activation(eu[:, :2 * nc2], ps[:, :2 * nc2],
                                         AF.Exp)
                    if J in (2, 3):
                        et = eT23[(gp, par, J)]
                    else:
                        et = eP.tile([128, 512], bf16, tag="et",
                                     name=f"et{gp}{par}_{J}")
                        _etile[(gp, par, J)] = et
                    if nc2 == 256:
                        nc.vector.tensor_tensor(et[:], eu[:],
                                                mk_sb[:], op=OP.mult)
                    else:                            # J=7: split per head
                        nc.vector.tensor_tensor(
                            et[:, 0:128], eu[:, 0:128], mk_sb[:, 0:128],
                            op=OP.mult)
                        nc.vector.tensor_tensor(
                            et[:, 256:384], eu[:, 128:256], mk_sb[:, 0:128],
                            op=OP.mult)

            def c_head_attn(h, half):
                """attnT fused 256-wide per j-block into PSUM (lazy zero)."""
                p, r0 = h // 2, 64 * (h % 2)
                dr = 32 * (h % 4)
                pb = pp.tile([65, 512], f32, tag="pb", bufs=2, name="pb")
                I0 = 4 * half
                first = True
                for J in range(max(0, I0 - 1), I0 + 4):
                    tl, off = et_tile(h, J)
                    ec0, el = off, 256
                    lo = 128 * (J - I0)
                    if J == I0 - 1:          # right half only (i-block I0)
                        ec0, el, lo = off + 128, 128, 0
                    elif J == I0 + 3:        # left half only (i-block I0+3)
                        el = 128
                    nc.tensor.matmul(
                        pb[:, lo:lo + el], v_sb[J][:, 0:65],
                        tl[:, ec0:ec0 + el],
                        start=first, stop=(J == I0 + 3),
                        skip_group_check=True)
                    first = False
                nc.vector.tensor_scalar_add(
                    dn[2 * half + h // 4][dr:dr + 1, :], pb[64:65, :],
                    es_sb[dr:dr + 1, (h // 4):(h // 4) + 1])
                nc.scalar.activation(atr[p][r0:r0 + 64,
                                            512 * half:512 * (half + 1)],
                                     pb[0:64, :], AF.Copy)

            def c_epilogue(half):
                with nc.allow_low_precision(reason="bf16 attn scale"):
                    for hg in range(2):
                        x = 2 * half + hg
                        nc.vector.reciprocal_approx_fast(rscr[:], dn[x][:])
                        nc.vector.tensor_copy(rdnb[x][:], rscr[:])
                for p in range(4):
                    prt = pst("prt")
                    nc.tensor.matmul(
                        prt[:], sel_sb[:, 128 * (p % 2):128 * (p % 2 + 1)],
                        rdnb[2 * half + p // 2][:], start=True, stop=True)
                    pc = eU.tile([128, 512], bf16, tag="prtc")
                    nc.scalar.activation(pc[:], prt[:], AF.Copy)
                    cs = slice(512 * half, 512 * (half + 1))
                    nc.vector.tensor_tensor(atf[p][:, cs], atr[p][:, cs],
                                            pc[:], op=OP.mult)

            def d_block(it):
                obt = eU.tile([128, DIM], bf16, tag="ob", bufs=3, name="obt")
                eng = nc.sync if it % 2 == 0 else nc.scalar
                alt = nc.scalar if it % 2 == 0 else nc.sync
                for dd in range(6):
                    po = pqt("po")
                    for et in range(4):
                        nc.tensor.matmul(
                            po[:, 0:DDC],
                            atf[et][:, 128 * it:128 * (it + 1)],
                            wo_sb[:, DIM * et + DDC * dd:
                                  DIM * et + DDC * (dd + 1)],
                            start=(et == 0), stop=(et == 3))
                    if dd % 2 == 0:
                        nc.scalar.activation(
                            obt[:, DDC * dd:DDC * (dd + 1)], po[:, 0:DDC],
                            AF.Copy)
                    else:
                        nc.vector.tensor_copy(
                            obt[:, DDC * dd:DDC * (dd + 1)], po[:, 0:DDC])
                    if it >= 6 and dd == 2:
                        eng.dma_start(
                            out_d[128 * it:128 * (it + 1), 0:3 * DDC],
                            obt[:, 0:3 * DDC])
                if it >= 6:
                    alt.dma_start(
                        out_d[128 * it:128 * (it + 1), 3 * DDC:DIM],
                        obt[:, 3 * DDC:DIM])
                else:
                    eng.dma_start(out_d[128 * it:128 * (it + 1), :], obt[:])

            # ---------------- Phase A + B + C-L (interleaved) -------------
            proj_kv_q0_sc(0)
            nc.scalar.dma_start(wq_sb[:, GQ:2 * GQ], wqg[:, GQ:2 * GQ])
            nc.sync.dma_start(wq_sb[:, 3 * GQ:4 * GQ], wqg[:, 3 * GQ:4 * GQ])
            proj_kv_q0_sc(1)
            nc.sync.dma_start(wq_sb[:, 2 * GQ:3 * GQ], wqg[:, 2 * GQ:3 * GQ])
            nc.sync.dma_start(pm_sb[:], perm[:])
            nc.sync.dma_start(id_sb[:], idm[:])
            nc.sync.dma_start(ck_sb[:], cosk[:])
            nc.sync.dma_start(sk_sb[:], sinkt[:])
            proj_group(qT[1], 1)
            nc.scalar.dma_start(cq_sb[:], cosq[:])
            nc.scalar.dma_start(sq_sb[:], sinq[:])
            nc.scalar.dma_start(mk_sb[:], mask01[:])
            nc.scalar.dma_start(sel_sb[:], sel2[:])
            nc.scalar.dma_start(es_sb[:], es2[:])
            nc.sync.dma_start(wo_sb[:, 0:2 * DIM], wog[:, 0:2 * DIM])
            v_transpose()
            rope(kr_sb, kv_sb, ck_sb, sk_sb, 64)
            nc.sync.dma_start(kr_sb[64:128, :], kr_sb[0:64, :])
            nc.scalar.dma_start(wo_sb[:, 2 * DIM:4 * DIM],
                                wog[:, 2 * DIM:4 * DIM])
            rope(qR[0], qT[0], cq_sb, sq_sb, 128)
            rope(qR[1], qT[1], cq_sb, sq_sb, 128)
            scores_pair(0, 0, 0, 0)
            scores_pair(0, 0, 0, 1)
            scores_pair(0, 1, 0, 0)
            scores_pair(0, 1, 0, 1)
            proj_group(qT[2], 2)
            rope(qR[2], qT[2], cq_sb, sq_sb, 128)
            c_head_attn(0, 0)
            c_head_attn(1, 0)
            proj_group(qT[3], 3)
            rope(qR[3], qT[3], cq_sb, sq_sb, 128)
            c_head_attn(2, 0)
            scores_pair(1, 0, 0, 0)
            scores_pair(1, 0, 0, 1)
            c_head_attn(3, 0)
            scores_pair(1, 1, 0, 0)
            scores_pair(1, 1, 0, 1)
            c_head_attn(4, 0)
            c_head_attn(5, 0)
            c_head_attn(6, 0)
            c_head_attn(7, 0)

            # ---------------- C-R interleaved with L-epi and D-left -------
            scores_pair(0, 0, 1, 0)
            scores_pair(0, 0, 1, 1)
            scores_pair(0, 1, 1, 0)
            scores_pair(0, 1, 1, 1)
            c_head_attn(0, 1)
            c_epilogue(0)
            c_head_attn(1, 1)
            d_block(0)
            c_head_attn(2, 1)
            c_head_attn(3, 1)
            scores_pair(1, 0, 1, 0)
            scores_pair(1, 0, 1, 1)
            d_block(1)
            c_head_attn(4, 1)
            scores_pair(1, 1, 1, 0)
            scores_pair(1, 1, 1, 1)
            c_head_attn(5, 1)
            d_block(2)
            c_head_attn(6, 1)
            c_head_attn(7, 1)
            c_epilogue(1)
            d_block(3)
            for it in range(4, NJ):
                d_block(it)

    nc.compile()
    return nc


def _host_prep(x, wq_w, wq_b, wk_w, wk_b, wv_w, wv_b, wo_w, wo_b, sinks):
    """Build per-core input maps (host-side sharding + bf16 layout prep)."""
    import ml_dtypes
    bf = ml_dtypes.bfloat16
    f = np.float32
    xT = np.ascontiguousarray(x.reshape(S, DIM).T).astype(f)   # [2880, 1024]
    xt = np.zeros((128, DT * S), f)
    for t in range(DT):
        dp = min(128, DIM - 128 * t)
        xt[:dp, S * t:S * (t + 1)] = xT[128 * t:128 * t + dp]
    xt[64, S * (DT - 1):] = 1.0                                # bias row
    xt = xt.astype(bf)

    half = HD // 2
    inv_freq = 1.0 / (THETA ** (np.arange(half, dtype=np.float64) * 2.0 / HD))
    ang = np.arange(S, dtype=np.float64)[:, None] * inv_freq   # [S, 32]
    cos_t = np.cos(ang).T.astype(f)                            # [32, S]
    sin_t = np.sin(ang).T.astype(f)
    cos64 = np.concatenate([cos_t, cos_t], 0)                  # [64, S]
    sin64 = np.concatenate([sin_t, sin_t], 0)                  # sign in perm
    scale = np.float32(HD ** -0.5)
    cosq = (np.concatenate([cos64, cos64], 0) * scale).astype(bf)
    sinq = (np.concatenate([sin64, sin64], 0) * scale).astype(bf)
    cosk = cos64.astype(bf)
    sinkt = sin64.astype(bf)

    # signed rotate-half permutation, as matmul lhsT: perm[src, a] = sign
    # out[a] = -in[a+32] for a%64<32 else +in[a-32]
    perm = np.zeros((128, 128), f)
    for a in range(128):
        if (a // 32) % 2 == 0:
            perm[a + 32, a] = -1.0
        else:
            perm[a - 32, a] = 1.0
    perm = perm.astype(bf)

    idm = np.zeros((128, 64), f)
    for i in range(64):
        idm[64 + i, i] = 1.0
    idm = idm.astype(bf)

    jj = np.arange(128)[:, None]
    ii = np.arange(512)[None, :]
    ib = ii % 256
    allow_l = (jj <= ib) & (ib < 128)
    allow_r = (ib >= 128) & (jj > ib - 128)
    mask01 = np.where(allow_l | allow_r, 1.0, 0.0).astype(bf)  # [128, 512]

    sel2 = np.zeros((128, 256), f)
    for s in range(2):                       # selA: rows 0,32; selB: 64,96
        sel2[64 * s, 128 * s:128 * s + 64] = 1.0
        sel2[64 * s + 32, 128 * s + 64:128 * (s + 1)] = 1.0
    sel2 = sel2.astype(bf)

    def tileT(w, b):
        # w [E, DIM] (+ bias b [E]) -> [128, DT*E] tiled transpose, bias@row64
        E = w.shape[0]
        o = np.zeros((128, DT * E), f)
        for t in range(DT):
            dp = min(128, DIM - 128 * t)
            o[:dp, E * t:E * (t + 1)] = w[:, 128 * t:128 * t + dp].T
        o[64, E * (DT - 1):] = b
        return o

    def esink_layout(s8):
        out = np.zeros((128, 2), f)
        for h in range(HL):
            out[32 * (h % 4), h // 4] = np.exp(np.float64(s8[h]))
        return out

    in_maps = []
    for c in range(NC):
        wq_c = wq_w[EL * c:EL * (c + 1)]                  # [512, 2880]
        wqb_c = wq_b[EL * c:EL * (c + 1)]
        wqg = np.zeros((128, 4 * DT * 128), f)
        for g in range(4):
            wqg[:, g * DT * 128:(g + 1) * DT * 128] = tileT(
                wq_c[128 * g:128 * (g + 1)], wqb_c[128 * g:128 * (g + 1)])
        wkv_c = np.concatenate([wk_w[HD * c:HD * (c + 1)],
                                wv_w[HD * c:HD * (c + 1)]], 0)
        wkvb_c = np.concatenate([wk_b[HD * c:HD * (c + 1)],
                                 wv_b[HD * c:HD * (c + 1)]])
        wo_c = np.ascontiguousarray(wo_w[:, EL * c:EL * (c + 1)].T)
        wog = np.zeros((128, 4 * DIM), f)
        for et in range(4):
            wog[:, DIM * et:DIM * (et + 1)] = wo_c[128 * et:128 * (et + 1)]
        in_maps.append({
            "xt": xt,
            "wqg": wqg.astype(bf),
            "wkvg": tileT(wkv_c, wkvb_c).astype(bf),
            "wog": wog.astype(bf),
            "cosq": cosq, "sinq": sinq, "cosk": cosk, "sinkt": sinkt,
            "perm": perm, "idm": idm, "mask01": mask01, "sel2": sel2,
            "es2": esink_layout(sinks[HL * c:HL * (c + 1)]),
        })
    return in_maps


def run_on_hw(inputs, trace=False, **kw):
    from concourse import bass_utils
    if "nc" not in _cache:
        _cache["nc"] = _build_module()
    in_maps = _host_prep(**inputs)
    res = bass_utils.run_bass_kernel_spmd(
        _cache["nc"], in_maps, core_ids=list(range(NC)), trace=trace, **kw)
    out = np.zeros((S, DIM), np.float64)
    for c in range(NC):
        out += res.results[c]["out"].astype(np.float64)
    out = (out + inputs["wo_b"].astype(np.float64)).astype(np.float32)
    return out.reshape(B, S, DIM), res


def kernel(**inputs) -> np.ndarray:
    out, _ = run_on_hw(inputs, trace=False)
    return out


# revision 56
# speedup vs baseline: 1.1721x; 1.0164x over previous
"""Sparse (sliding-window + sink) GQA attention block on 8 TRN2 NeuronCores.

v8: full-bf16 matmul operands; streamed x with graduated per-chunk DMA
deps; weights-first two-queue schedule with DMA issues interleaved into
the compute emission (so the 2-deep per-queue issue throttle never
blocks the ACT drain stream); PE-based v transpose; paired-head scores
matmuls (one 512-wide matmul covers two heads' windows at a j-block);
fused 256-wide attention matmuls (PSUM lazy-zero); split tail out-DMA.

Sharding: tensor-parallel over the 64 q-heads -> 8 q-heads (= 1 kv-head
group) per core; x replicated; wo partial outputs summed on host.

Per-core dataflow:
  A:  qkv projections; kv+q0 interleaved per x-tile sc-outer so ACT
      drains pipeline; x resident in SBUF (23 bf16 tiles); biases baked
      as an extra contraction row.  Warmup matmuls on a zero tile spin
      the PE clock gate up while inputs stream in.
  B:  RoPE rotate-half via a signed permutation matmul on the PE, then
      bf16 DVE mults; 0.125 score scale baked into the q-side cos/sin
      tables; v transposed on the PE against an identity rhs.
  C:  two half-passes (i<512, i>=512), software-pipelined across heads
      and interleaved under A / D.  Per head-half: scoresT into PSUM,
      exp on ACT, 0/1-mask multiply on DVE (bf16), attnT accumulated
      with fused 256-wide matmuls per j-block directly in PSUM.
      Denominators via the v ones-row; reciprocal_approx_fast; per-pair
      broadcast via a selector matmul; bf16 scale.
  D:  out[i,dd] partials (it-blocks 0-3 interleaved with C's second
      half), per-it batched bf16 DMA out alternating queues; the last
      two blocks split their DMA across both queues.
"""

import numpy as np

B, S, DIM = 1, 1024, 2880
H, HKV, HD = 64, 8, 64
GROUP = H // HKV
WINDOW = 128
THETA = 150000.0
NC = 8
HL = H // NC                 # 8 local q-heads per core
EL = HL * HD                 # 512 local q-dim
DT = (DIM + 127) // 128      # 23 d-tiles (22 full + 64)
NJ = S // 128                # 8 j/i blocks
DDC = 480                    # out-proj column chunk (6 per row-block)

_cache = {}


def _build_module():
    import concourse.bacc as bacc
    import concourse.mybir as mybir
    import concourse.tile as tile

    f32 = mybir.dt.float32
    bf16 = mybir.dt.bfloat16
    AF = mybir.ActivationFunctionType
    OP = mybir.AluOpType

    nc = bacc.Bacc("TRN2", target_bir_lowering=False, debug=False)

    def din(name, shape, dt=bf16):
        return nc.dram_tensor(name, shape, dt, kind="ExternalInput").ap()

    xt = din("xt", [128, DT * S])            # x^T tiled; tile22 row64 = 1.0
    wqg = din("wqg", [128, 4 * DT * 128])    # [(g*23+t)*128+e]; bias row64@t22
    wkvg = din("wkvg", [128, DT * 128])      # k|v; bias row64@t22
    wog = din("wog", [128, 4 * DIM])         # [2880*et + dd]
    cosq = din("cosq", [128, S])             # 0.125-scaled
    sinq = din("sinq", [128, S])             # 0.125-scaled (sign in perm)
    cosk = din("cosk", [64, S])
    sinkt = din("sinkt", [64, S])
    perm = din("perm", [128, 128])           # signed rotate-half permutation
    idm = din("idm", [128, 64])              # I64 at rows 64-127 (v transp.)
    mask01 = din("mask01", [128, 512])       # 0/1 window mask, 2 j-blocks
    sel2 = din("sel2", [128, 256])           # selA | selB
    es2 = din("es2", [128, 2], f32)          # exp(sinks), row 32*(h%4)
    out_d = nc.dram_tensor("out", [S, DIM], bf16, kind="ExternalOutput").ap()

    # x chunk boundaries (tile indices); graduated sizes, alternate queues
    XCH = [(0, 1), (1, 2), (2, 4), (4, 8), (8, 12), (12, 16), (16, 20),
           (20, 23)]

    with tile.TileContext(nc) as tc:
        import contextlib
        with contextlib.ExitStack() as ctx:
            res = ctx.enter_context(tc.tile_pool(name="res", bufs=1))
            x_ch = [res.tile([128, (b - a) * S], bf16, tag=f"xc{i}",
                             name=f"xc{i}")
                    for i, (a, b) in enumerate(XCH)]
            x_sb = []
            for i, (a, b) in enumerate(XCH):
                for t in range(a, b):
                    x_sb.append(x_ch[i][:, S * (t - a):S * (t - a + 1)])
            wq_sb = res.tile([128, 4 * DT * 128], bf16, tag="wq")
            wkv_sb = res.tile([128, DT * 128], bf16, tag="wkv")
            wo_sb = res.tile([128, 4 * DIM], bf16, tag="wo")
            cq_sb = res.tile([128, S], bf16, tag="cq")
            sq_sb = res.tile([128, S], bf16, tag="sq")
            ck_sb = res.tile([64, S], bf16, tag="ck")
            sk_sb = res.tile([64, S], bf16, tag="sk")
            pm_sb = res.tile([128, 128], bf16, tag="pm")
            id_sb = res.tile([128, 64], bf16, tag="idm")
            mk_sb = res.tile([128, 512], bf16, tag="mk")
            sel_sb = res.tile([128, 256], bf16, tag="sel")
            es_sb = res.tile([128, 2], f32, tag="es")
            ones0 = res.tile([128, 2], bf16, tag="ones0")
            kv_sb = res.tile([128, S], bf16, tag="kv")
            kr_sb = res.tile([128, S], bf16, tag="kr")
            v_sb = [res.tile([128, 65], bf16, tag=f"v{j}", name=f"v{j}")
                    for j in range(NJ)]
            qT = [res.tile([128, S], bf16, tag=f"qT{g}", name=f"qT{g}")
                  for g in range(4)]
            # qR merged per group-pair so one scores matmul spans 2 heads
            qRp = [res.tile([128, 2, S], bf16, tag=f"qRp{gp}", name=f"qRp{gp}")
                   for gp in range(2)]
            qR = [qRp[g // 2][:, g % 2, :] for g in range(4)]
            # persistent e^T tiles for j-blocks 2,3 (used by both halves)
            eT23 = {(gp, par, J): res.tile([128, 512], bf16,
                                           tag=f"e23_{gp}{par}{J}",
                                           name=f"e23_{gp}{par}{J}")
                    for gp in range(2) for par in range(2) for J in (2, 3)}
            atr = [res.tile([128, S], bf16, tag=f"atr{p}", name=f"atr{p}")
                   for p in range(4)]
            atf = [res.tile([128, S], bf16, tag=f"atf{p}", name=f"atf{p}")
                   for p in range(4)]
            # dn[2*half + hg]: heads 4*hg..4*hg+3 at partitions 0/32/64/96
            dn = [res.tile([128, 512], f32, tag=f"dn{x}", name=f"dn{x}")
                  for x in range(4)]
            rdnb = [res.tile([128, 512], bf16, tag=f"rdb{x}", name=f"rdb{x}")
                    for x in range(4)]
            rscr = res.tile([128, 512], f32, tag="rscr")
            wrm = res.tile([128, 512], bf16, tag="wrm")

            # ------- resident DMAs: weights-first, x streamed per chunk ---
            GQ = DT * 128

            def dma_xch(eng, i):
                a, b = XCH[i]
                eng.dma_start(x_ch[i][:], xt[:, S * a:S * b])

            # Only the early-needed inputs are issued up front; the rest are
            # issued mid-program, interleaved with ACT/SP work, so a DMA
            # issue never blocks the ACT compute stream at the 2-deep
            # per-queue throttle.
            nc.sync.dma_start(wkv_sb[:], wkvg[:])
            nc.scalar.dma_start(wq_sb[:, 0:GQ], wqg[:, 0:GQ])
            dma_xch(nc.sync, 0)
            dma_xch(nc.scalar, 1)
            dma_xch(nc.sync, 2)
            dma_xch(nc.scalar, 3)
            dma_xch(nc.sync, 4)
            dma_xch(nc.scalar, 5)
            dma_xch(nc.sync, 6)
            dma_xch(nc.scalar, 7)
            nc.vector.memset(wrm[:], 0.0)
            nc.vector.memset(ones0[:], 1.0)
            for x in range(4):
                nc.vector.memset(dn[x][:], 1.0)

            pp = ctx.enter_context(
                tc.tile_pool(name="pp", bufs=2, space="PSUM"))
            rp = ctx.enter_context(tc.tile_pool(name="rp", bufs=2))
            eU = ctx.enter_context(tc.tile_pool(name="eU", bufs=3))
            eP = ctx.enter_context(tc.tile_pool(name="eP", bufs=8))

            _etile = {}

            def pqt(name):
                return pp.tile([128, 512], f32, tag="pq", bufs=3, name=name)

            def pst(name):
                return pp.tile([128, 512], f32, tag="ps", bufs=2, name=name)

            def warm1():
                pw = pst("pw")
                nc.tensor.matmul(pw[:], wrm[:, 0:128], wrm[:],
                                 start=True, stop=True)

            # ------- PE warmup: spin the HAM up while inputs stream -------
            for wi in range(15):
                warm1()

            # ---------------- helpers ----------------
            def proj_kv_q0_sc(sc):
                """kv + q0 over x-tiles for one sc half; ACT drains after.

                On the sc0 (DMA-paced) pass, a warmup matmul is inserted at
                each x-chunk boundary to keep the PE p-state hot while the
                next chunk lands.
                """
                ak = pqt("pak")
                aq = pqt("paq")
                for t in range(DT):
                    if sc == 0 and t in (4, 8, 12, 16, 20):
                        warm1()
                    dp = 128 if t < DT - 1 else DIM - 128 * (DT - 1) + 1
                    st, sp = (t == 0), (t == DT - 1)
                    nc.tensor.matmul(
                        ak[:], wkv_sb[:dp, 128 * t:128 * (t + 1)],
                        x_sb[t][:dp, 512 * sc:512 * (sc + 1)],
                        start=st, stop=sp)
                    nc.tensor.matmul(
                        aq[:], wq_sb[:dp, 128 * t:128 * (t + 1)],
                        x_sb[t][:dp, 512 * sc:512 * (sc + 1)],
                        start=st, stop=sp)
                nc.scalar.activation(kv_sb[:, 512 * sc:512 * (sc + 1)],
                                     ak[:], AF.Copy)
                nc.scalar.activation(qT[0][:, 512 * sc:512 * (sc + 1)],
                                     aq[:], AF.Copy)

            def v_transpose():
                """vT via PE transpose (identity rhs); append ones column."""
                for j in range(NJ):
                    pvt = pp.tile([128, 64], bf16, tag="pv", bufs=1,
                                  name="pv")
                    pv = pvt[:]
                    nc.tensor.matmul(
                        pv, kv_sb[64:128, 128 * j:128 * (j + 1)],
                        id_sb[64:128, 0:64],
                        start=True, stop=True, is_transpose=True)
                    nc.scalar.activation(v_sb[j][:, 0:64], pv, AF.Copy)
                    nc.vector.tensor_copy(v_sb[j][:, 64:65], ones0[:, 0:1])

            def proj_group(dst, g):
                """2x23 matmuls (ap=512) into ping-pong psums, ACT drains."""
                for sc in range(2):
                    pq = pqt("pq")
                    for t in range(DT):
                        dp = 128 if t < DT - 1 else DIM - 128 * (DT - 1) + 1
                        nc.tensor.matmul(
                            pq[:], wq_sb[:dp, (g * DT + t) * 128:
                                         (g * DT + t + 1) * 128],
                            x_sb[t][:dp, 512 * sc:512 * (sc + 1)],
                            start=(t == 0), stop=(t == DT - 1))
                    nc.scalar.activation(dst[:, 512 * sc:512 * (sc + 1)],
                                         pq[:], AF.Copy)

            def rope(dst, src, cos, sin, npart):
                """dst = src*cos + perm(src)*sin via PE perm + DVE mults."""
                for half in range(2):
                    cs = slice(512 * half, 512 * (half + 1))
                    psw = pqt("psw")
                    nc.tensor.matmul(psw[:npart, :], pm_sb[:npart, :npart],
                                     src[:npart, cs], start=True, stop=True)
                    tmp = rp.tile([128, 512], bf16, tag="tmp")
                    qc = rp.tile([128, 512], bf16, tag="qc")
                    nc.vector.tensor_tensor(tmp[:npart], psw[:npart, :],
                                            sin[:npart, cs], op=OP.mult)
                    nc.vector.tensor_tensor(qc[:npart], src[:npart, cs],
                                            cos[:npart, cs], op=OP.mult)
                    nc.vector.tensor_tensor(dst[:npart, cs], qc[:npart],
                                            tmp[:npart], op=OP.add)

            def et_tile(h, J):
                """-> (tile, col offset) of head h's 256-span at j-block J."""
                gp, par = h // 4, h % 2
                off = ((h // 2) % 2) * 256
                if J in (2, 3):
                    return eT23[(gp, par, J)], off
                return _etile[(gp, par, J)], off

            def scores_pair(gp, par, half, jp):
                """scores -> exp -> mask for 2 heads (groups 2gp, 2gp+1) of
                one row parity, 2 j-blocks.  One 512-wide matmul per
                j-block spans both heads' 256-col i-windows."""
                r0 = 64 * par
                for J in range(4 * half + 2 * jp, 4 * half + 2 * jp + 2):
                    ilo = 128 * J
                    nc2 = min(256, S - ilo)          # J=7 span is 128
                    ps = pst("ps")
                    nc.tensor.matmul(
                        ps[:, 0:2 * nc2],
                        kr_sb[r0:r0 + 64, 128 * J:128 * (J + 1)],
                        qRp[gp][r0:r0 + 64, 0:2, ilo:ilo + nc2],
                        start=True, stop=True)
                    eu = eU.tile([128, 512], bf16, tag="eu")
                    nc.scalar.activation(eu[:, :2 * nc2], ps[:, :2 * nc2],
                                         AF.Exp)
                    if J in (2, 3):
                        et = eT23[(gp, par, J)]
                    else:
                        et = eP.tile([128, 512], bf16, tag="et",
                                     name=f"et{gp}{par}_{J}")
                        _etile[(gp, par, J)] = et
                    if nc2 == 256:
                        nc.vector.tensor_tensor(et[:], eu[:],
                                                mk_sb[:], op=OP.mult)
                    else:                            # J=7: split per head
                        nc.vector.tensor_tensor(
                            et[:, 0:128], eu[:, 0:128], mk_sb[:, 0:128],
                            op=OP.mult)
                        nc.vector.tensor_tensor(
                            et[:, 256:384], eu[:, 128:256], mk_sb[:, 0:128],
                            op=OP.mult)

            def c_head_attn(h, half):
                """attnT fused 256-wide per j-block into PSUM (lazy zero)."""
                p, r0 = h // 2, 64 * (h % 2)
                dr = 32 * (h % 4)
                pb = pp.tile([65, 512], f32, tag="pb", bufs=2, name="pb")
                I0 = 4 * half
                first = True
                for J in range(max(0, I0 - 1), I0 + 4):
                    tl, off = et_tile(h, J)
                    ec0, el = off, 256
                    lo = 128 * (J - I0)
                    if J == I0 - 1:          # right half only (i-block I0)
                        ec0, el, lo = off + 128, 128, 0
                    elif J == I0 + 3:        # left half only (i-block I0+3)
                        el = 128
                    nc.tensor.matmul(
                        pb[:, lo:lo + el], v_sb[J][:, 0:65],
                        tl[:, ec0:ec0 + el],
                        start=first, stop=(J == I0 + 3),
                        skip_group_check=True)
                    first = False
                nc.vector.tensor_scalar_add(
                    dn[2 * half + h // 4][dr:dr + 1, :], pb[64:65, :],
                    es_sb[dr:dr + 1, (h // 4):(h // 4) + 1])
                nc.scalar.activation(atr[p][r0:r0 + 64,
                                            512 * half:512 * (half + 1)],
                                     pb[0:64, :], AF.Copy)

            def c_epilogue(half):
                with nc.allow_low_precision(reason="bf16 attn scale"):
                    for hg in range(2):
                        x = 2 * half + hg
                        nc.vector.reciprocal_approx_fast(rscr[:], dn[x][:])
                        nc.vector.tensor_copy(rdnb[x][:], rscr[:])
                for p in range(4):
                    prt = pst("prt")
                    nc.tensor.matmul(
                        prt[:], sel_sb[:, 128 * (p % 2):128 * (p % 2 + 1)],
                        rdnb[2 * half + p // 2][:], start=True, stop=True)
                    pc = eU.tile([128, 512], bf16, tag="prtc")
                    nc.scalar.activation(pc[:], prt[:], AF.Copy)
                    cs = slice(512 * half, 512 * (half + 1))
                    nc.vector.tensor_tensor(atf[p][:, cs], atr[p][:, cs],
                                            pc[:], op=OP.mult)

            def d_block(it):
                obt = eU.tile([128, DIM], bf16, tag="ob", bufs=3, name="obt")
                eng = nc.sync if it % 2 == 0 else nc.scalar
                alt = nc.scalar if it % 2 == 0 else nc.sync
                for dd in range(6):
                    po = pqt("po")
                    for et in range(4):
                        nc.tensor.matmul(
                            po[:, 0:DDC],
                            atf[et][:, 128 * it:128 * (it + 1)],
                            wo_sb[:, DIM * et + DDC * dd:
                                  DIM * et + DDC * (dd + 1)],
                            start=(et == 0), stop=(et == 3))
                    if dd % 2 == 0:
                        nc.scalar.activation(
                            obt[:, DDC * dd:DDC * (dd + 1)], po[:, 0:DDC],
                            AF.Copy)
                    else:
                        nc.vector.tensor_copy(
                            obt[:, DDC * dd:DDC * (dd + 1)], po[:, 0:DDC])
                    if it >= 6 and dd == 2:
                        eng.dma_start(
                            out_d[128 * it:128 * (it + 1), 0:3 * DDC],
                            obt[:, 0:3 * DDC])
                if it >= 6:
                    alt.dma_start(
                        out_d[128 * it:128 * (it + 1), 3 * DDC:DIM],
                        obt[:, 3 * DDC:DIM])
                else:
                    eng.dma_start(out_d[128 * it:128 * (it + 1), :], obt[:])

            # ---------------- Phase A + B + C-L (interleaved) -------------
            proj_kv_q0_sc(0)
            nc.scalar.dma_start(wq_sb[:, GQ:2 * GQ], wqg[:, GQ:2 * GQ])
            nc.sync.dma_start(wq_sb[:, 3 * GQ:4 * GQ], wqg[:, 3 * GQ:4 * GQ])
            proj_kv_q0_sc(1)
            nc.sync.dma_start(wq_sb[:, 2 * GQ:3 * GQ], wqg[:, 2 * GQ:3 * GQ])
            nc.sync.dma_start(pm_sb[:], perm[:])
            nc.sync.dma_start(id_sb[:], idm[:])
            nc.sync.dma_start(ck_sb[:], cosk[:])
            nc.sync.dma_start(sk_sb[:], sinkt[:])
            proj_group(qT[1], 1)
            nc.scalar.dma_start(cq_sb[:], cosq[:])
            nc.scalar.dma_start(sq_sb[:], sinq[:])
            nc.scalar.dma_start(mk_sb[:], mask01[:])
            nc.scalar.dma_start(sel_sb[:], sel2[:])
            nc.scalar.dma_start(es_sb[:], es2[:])
            nc.sync.dma_start(wo_sb[:, 0:2 * DIM], wog[:, 0:2 * DIM])
            v_transpose()
            rope(kr_sb, kv_sb, ck_sb, sk_sb, 64)
            nc.sync.dma_start(kr_sb[64:128, :], kr_sb[0:64, :])
            nc.scalar.dma_start(wo_sb[:, 2 * DIM:4 * DIM],
                                wog[:, 2 * DIM:4 * DIM])
            rope(qR[0], qT[0], cq_sb, sq_sb, 128)
            rope(qR[1], qT[1], cq_sb, sq_sb, 128)
            scores_pair(0, 0, 0, 0)
            scores_pair(0, 0, 0, 1)
            scores_pair(0, 1, 0, 0)
            scores_pair(0, 1, 0, 1)
            proj_group(qT[2], 2)
            rope(qR[2], qT[2], cq_sb, sq_sb, 128)
            c_head_attn(0, 0)
            c_head_attn(1, 0)
            proj_group(qT[3], 3)
            rope(qR[3], qT[3], cq_sb, sq_sb, 128)
            c_head_attn(2, 0)
            scores_pair(1, 0, 0, 0)
            scores_pair(1, 0, 0, 1)
            c_head_attn(3, 0)
            scores_pair(1, 1, 0, 0)
            scores_pair(1, 1, 0, 1)
            c_head_attn(4, 0)
            c_head_attn(5, 0)
            c_head_attn(6, 0)
            c_head_attn(7, 0)

            # ---------------- C-R interleaved with L-epi and D-left -------
            scores_pair(0, 0, 1, 0)
            scores_pair(0, 0, 1, 1)
            scores_pair(0, 1, 1, 0)
            scores_pair(0, 1, 1, 1)
            c_head_attn(0, 1)
            c_epilogue(0)
            c_head_attn(1, 1)
            d_block(0)
            c_head_attn(2, 1)
            c_head_attn(3, 1)
            scores_pair(1, 0, 1, 0)
            scores_pair(1, 0, 1, 1)
            d_block(1)
            c_head_attn(4, 1)
            scores_pair(1, 1, 1, 0)
            scores_pair(1, 1, 1, 1)
            c_head_attn(5, 1)
            d_block(2)
            c_head_attn(6, 1)
            c_head_attn(7, 1)
            c_epilogue(1)
            d_block(3)
            for it in range(4, NJ):
                d_block(it)

    nc.compile()
    return nc


def _host_prep(x, wq_w, wq_b, wk_w, wk_b, wv_w, wv_b, wo_w, wo_b, sinks):
    """Build per-core input maps (host-side sharding + bf16 layout prep)."""
    import ml_dtypes
    bf = ml_dtypes.bfloat16
    f = np.float32
    xT = np.ascontiguousarray(x.reshape(S, DIM).T).astype(f)   # [2880, 1024]
    xt = np.zeros((128, DT * S), f)
    for t in range(DT):
        dp = min(128, DIM - 128 * t)
        xt[:dp, S * t:S * (t + 1)] = xT[128 * t:128 * t + dp]
    xt[64, S * (DT - 1):] = 1.0                                # bias row
    xt = xt.astype(bf)

    half = HD // 2
    inv_freq = 1.0 / (THETA ** (np.arange(half, dtype=np.float64) * 2.0 / HD))
    ang = np.arange(S, dtype=np.float64)[:, None] * inv_freq   # [S, 32]
    cos_t = np.cos(ang).T.astype(f)                            # [32, S]
    sin_t = np.sin(ang).T.astype(f)
    cos64 = np.concatenate([cos_t, cos_t], 0)                  # [64, S]
    sin64 = np.concatenate([sin_t, sin_t], 0)                  # sign in perm
    scale = np.float32(HD ** -0.5)
    cosq = (np.concatenate([cos64, cos64], 0) * scale).astype(bf)
    sinq = (np.concatenate([sin64, sin64], 0) * scale).astype(bf)
    cosk = cos64.astype(bf)
    sinkt = sin64.astype(bf)

    # signed rotate-half permutation, as matmul lhsT: perm[src, a] = sign
    # out[a] = -in[a+32] for a%64<32 else +in[a-32]
    perm = np.zeros((128, 128), f)
    for a in range(128):
        if (a // 32) % 2 == 0:
            perm[a + 32, a] = -1.0
        else:
            perm[a - 32, a] = 1.0
    perm = perm.astype(bf)

    idm = np.zeros((128, 64), f)
    for i in range(64):
        idm[64 + i, i] = 1.0
    idm = idm.astype(bf)

    jj = np.arange(128)[:, None]
    ii = np.arange(512)[None, :]
    ib = ii % 256
    allow_l = (jj <= ib) & (ib < 128)
    allow_r = (ib >= 128) & (jj > ib - 128)
    mask01 = np.where(allow_l | allow_r, 1.0, 0.0).astype(bf)  # [128, 512]

    sel2 = np.zeros((128, 256), f)
    for s in range(2):                       # selA: rows 0,32; selB: 64,96
        sel2[64 * s, 128 * s:128 * s + 64] = 1.0
        sel2[64 * s + 32, 128 * s + 64:128 * (s + 1)] = 1.0
    sel2 = sel2.astype(bf)

    def tileT(w, b):
        # w [E, DIM] (+ bias b [E]) -> [128, DT*E] tiled transpose, bias@row64
        E = w.shape[0]
        o = np.zeros((128, DT * E), f)
        for t in range(DT):
            dp = min(128, DIM - 128 * t)
            o[:dp, E * t:E * (t + 1)] = w[:, 128 * t:128 * t + dp].T
        o[64, E * (DT - 1):] = b
        return o

    def esink_layout(s8):
        out = np.zeros((128, 2), f)
        for h in range(HL):
            out[32 * (h % 4), h // 4] = np.exp(np.float64(s8[h]))
        return out

    in_maps = []
    for c in range(NC):
        wq_c = wq_w[EL * c:EL * (c + 1)]                  # [512, 2880]
        wqb_c = wq_b[EL * c:EL * (c + 1)]
        wqg = np.zeros((128, 4 * DT * 128), f)
        for g in range(4):
            wqg[:, g * DT * 128:(g + 1) * DT * 128] = tileT(
                wq_c[128 * g:128 * (g + 1)], wqb_c[128 * g:128 * (g + 1)])
        wkv_c = np.concatenate([wk_w[HD * c:HD * (c + 1)],
                                wv_w[HD * c:HD * (c + 1)]], 0)
        wkvb_c = np.concatenate([wk_b[HD * c:HD * (c + 1)],
                                 wv_b[HD * c:HD * (c + 1)]])
        wo_c = np.ascontiguousarray(wo_w[:, EL * c:EL * (c + 1)].T)
        wog = np.zeros((128, 4 * DIM), f)
        for et in range(4):
            wog[:, DIM * et:DIM * (et + 1)] = wo_c[128 * et:128 * (et + 1)]
        in_maps.append({
            "xt": xt,
            "wqg": wqg.astype(bf),
            "wkvg": tileT(wkv_c, wkvb_c).astype(bf),
            "wog": wog.astype(bf),
            "cosq": cosq, "sinq": sinq, "cosk": cosk, "sinkt": sinkt,
            "perm": perm, "idm": idm, "mask01": mask01, "sel2": sel2,
            "es2": esink_layout(sinks[HL * c:HL * (c + 1)]),
        })
    return in_maps


def run_on_hw(inputs, trace=False, **kw):
    from concourse import bass_utils
    if "nc" not in _cache:
        _cache["nc"] = _build_module()
    in_maps = _host_prep(**inputs)
    res = bass_utils.run_bass_kernel_spmd(
        _cache["nc"], in_maps, core_ids=list(range(NC)), trace=trace, **kw)
    out = np.zeros((S, DIM), np.float64)
    for c in range(NC):
        out += res.results[c]["out"].astype(np.float64)
    out = (out + inputs["wo_b"].astype(np.float64)).astype(np.float32)
    return out.reshape(B, S, DIM), res


def kernel(**inputs) -> np.ndarray:
    out, _ = run_on_hw(inputs, trace=False)
    return out


# revision 57
# speedup vs baseline: 1.1948x; 1.0194x over previous
"""Sparse (sliding-window + sink) GQA attention block on 8 TRN2 NeuronCores.

v8: full-bf16 matmul operands; streamed x with graduated per-chunk DMA
deps; weights-first two-queue schedule with DMA issues interleaved into
the compute emission (so the 2-deep per-queue issue throttle never
blocks the ACT drain stream); PE-based v transpose; paired-head scores
matmuls (one 512-wide matmul covers two heads' windows at a j-block);
fused 256-wide attention matmuls (PSUM lazy-zero); split tail out-DMA.

Sharding: tensor-parallel over the 64 q-heads -> 8 q-heads (= 1 kv-head
group) per core; x replicated; wo partial outputs summed on host.

Per-core dataflow:
  A:  qkv projections; kv+q0 interleaved per x-tile sc-outer so ACT
      drains pipeline; x resident in SBUF (23 bf16 tiles); biases baked
      as an extra contraction row.  Warmup matmuls on a zero tile spin
      the PE clock gate up while inputs stream in.
  B:  RoPE rotate-half via a signed permutation matmul on the PE, then
      bf16 DVE mults; 0.125 score scale baked into the q-side cos/sin
      tables; v transposed on the PE against an identity rhs.
  C:  two half-passes (i<512, i>=512), software-pipelined across heads
      and interleaved under A / D.  Per head-half: scoresT into PSUM,
      exp on ACT, 0/1-mask multiply on DVE (bf16), attnT accumulated
      with fused 256-wide matmuls per j-block directly in PSUM.
      Denominators via the v ones-row; reciprocal_approx_fast; per-pair
      broadcast via a selector matmul; bf16 scale.
  D:  out[i,dd] partials (it-blocks 0-3 interleaved with C's second
      half), per-it batched bf16 DMA out alternating queues; the last
      two blocks split their DMA across both queues.
"""

import numpy as np

B, S, DIM = 1, 1024, 2880
H, HKV, HD = 64, 8, 64
GROUP = H // HKV
WINDOW = 128
THETA = 150000.0
NC = 8
HL = H // NC                 # 8 local q-heads per core
EL = HL * HD                 # 512 local q-dim
DT = (DIM + 127) // 128      # 23 d-tiles (22 full + 64)
NJ = S // 128                # 8 j/i blocks
DDC = 480                    # out-proj column chunk (6 per row-block)

_cache = {}


def _build_module():
    import concourse.bacc as bacc
    import concourse.mybir as mybir
    import concourse.tile as tile

    f32 = mybir.dt.float32
    bf16 = mybir.dt.bfloat16
    AF = mybir.ActivationFunctionType
    OP = mybir.AluOpType

    nc = bacc.Bacc("TRN2", target_bir_lowering=False, debug=False)

    def din(name, shape, dt=bf16):
        return nc.dram_tensor(name, shape, dt, kind="ExternalInput").ap()

    xt = din("xt", [128, DT * S])            # x^T tiled; tile22 row64 = 1.0
    wqg = din("wqg", [128, 4 * DT * 128])    # [(g*23+t)*128+e]; bias row64@t22
    wkvg = din("wkvg", [128, DT * 128])      # k|v; bias row64@t22
    wog = din("wog", [128, 4 * DIM])         # [2880*et + dd]
    cosq = din("cosq", [128, S])             # 0.125-scaled
    sinq = din("sinq", [128, S])             # 0.125-scaled (sign in perm)
    cosk = din("cosk", [64, S])
    sinkt = din("sinkt", [64, S])
    perm = din("perm", [128, 128])           # signed rotate-half permutation
    idm = din("idm", [128, 64])              # I64 at rows 64-127 (v transp.)
    mask01 = din("mask01", [128, 512])       # 0/1 window mask, 2 j-blocks
    sel2 = din("sel2", [128, 256])           # selA | selB
    es2 = din("es2", [128, 2], f32)          # exp(sinks), row 32*(h%4)
    out_d = nc.dram_tensor("out", [S, DIM], bf16, kind="ExternalOutput").ap()

    # x chunk boundaries (tile indices); graduated sizes, alternate queues
    XCH = [(0, 1), (1, 2), (2, 4), (4, 8), (8, 12), (12, 16), (16, 20),
           (20, 23)]

    with tile.TileContext(nc) as tc:
        import contextlib
        with contextlib.ExitStack() as ctx:
            res = ctx.enter_context(tc.tile_pool(name="res", bufs=1))
            x_ch = [res.tile([128, (b - a) * S], bf16, tag=f"xc{i}",
                             name=f"xc{i}")
                    for i, (a, b) in enumerate(XCH)]
            x_sb = []
            for i, (a, b) in enumerate(XCH):
                for t in range(a, b):
                    x_sb.append(x_ch[i][:, S * (t - a):S * (t - a + 1)])
            wq_sb = res.tile([128, 4 * DT * 128], bf16, tag="wq")
            wkv_sb = res.tile([128, DT * 128], bf16, tag="wkv")
            wo_sb = res.tile([128, 4 * DIM], bf16, tag="wo")
            cq_sb = res.tile([128, S], bf16, tag="cq")
            sq_sb = res.tile([128, S], bf16, tag="sq")
            ck_sb = res.tile([64, S], bf16, tag="ck")
            sk_sb = res.tile([64, S], bf16, tag="sk")
            pm_sb = res.tile([128, 128], bf16, tag="pm")
            id_sb = res.tile([128, 64], bf16, tag="idm")
            mk_sb = res.tile([128, 512], bf16, tag="mk")
            sel_sb = res.tile([128, 256], bf16, tag="sel")
            es_sb = res.tile([128, 2], f32, tag="es")
            ones0 = res.tile([128, 2], bf16, tag="ones0")
            kv_sb = res.tile([128, S], bf16, tag="kv")
            kr_sb = res.tile([128, S], bf16, tag="kr")
            v_sb = [res.tile([128, 65], bf16, tag=f"v{j}", name=f"v{j}")
                    for j in range(NJ)]
            qT = [res.tile([128, S], bf16, tag=f"qT{g}", name=f"qT{g}")
                  for g in range(4)]
            # qR merged per group-pair so one scores matmul spans 2 heads
            qRp = [res.tile([128, 2, S], bf16, tag=f"qRp{gp}", name=f"qRp{gp}")
                   for gp in range(2)]
            qR = [qRp[g // 2][:, g % 2, :] for g in range(4)]
            # persistent e^T tiles for j-blocks 2,3 (used by both halves)
            eT23 = {(gp, par, J): res.tile([128, 512], bf16,
                                           tag=f"e23_{gp}{par}{J}",
                                           name=f"e23_{gp}{par}{J}")
                    for gp in range(2) for par in range(2) for J in (2, 3)}
            atr = [res.tile([128, S], bf16, tag=f"atr{p}", name=f"atr{p}")
                   for p in range(4)]
            atf = [res.tile([128, S], bf16, tag=f"atf{p}", name=f"atf{p}")
                   for p in range(4)]
            # dn[2*half + hg]: heads 4*hg..4*hg+3 at partitions 0/32/64/96
            dn = [res.tile([128, 512], f32, tag=f"dn{x}", name=f"dn{x}")
                  for x in range(4)]
            rdnb = [res.tile([128, 512], bf16, tag=f"rdb{x}", name=f"rdb{x}")
                    for x in range(4)]
            rscr = res.tile([128, 512], f32, tag="rscr")
            wrm = res.tile([128, 512], bf16, tag="wrm")

            # ------- resident DMAs: weights-first, x streamed per chunk ---
            GQ = DT * 128

            def dma_xch(eng, i):
                a, b = XCH[i]
                eng.dma_start(x_ch[i][:], xt[:, S * a:S * b])

            # Only the early-needed inputs are issued up front; the rest are
            # issued mid-program, interleaved with ACT/SP work, so a DMA
            # issue never blocks the ACT compute stream at the 2-deep
            # per-queue throttle.
            nc.sync.dma_start(wkv_sb[:], wkvg[:])
            nc.scalar.dma_start(wq_sb[:, 0:GQ], wqg[:, 0:GQ])
            dma_xch(nc.sync, 0)
            dma_xch(nc.scalar, 1)
            dma_xch(nc.sync, 2)
            dma_xch(nc.scalar, 3)
            dma_xch(nc.sync, 4)
            dma_xch(nc.scalar, 5)
            dma_xch(nc.sync, 6)
            dma_xch(nc.scalar, 7)
            nc.vector.memset(wrm[:], 0.0)
            nc.vector.memset(ones0[:], 1.0)
            for x in range(4):
                nc.vector.memset(dn[x][:], 1.0)

            pp = ctx.enter_context(
                tc.tile_pool(name="pp", bufs=2, space="PSUM"))
            rp = ctx.enter_context(tc.tile_pool(name="rp", bufs=2))
            eU = ctx.enter_context(tc.tile_pool(name="eU", bufs=3))
            eP = ctx.enter_context(tc.tile_pool(name="eP", bufs=8))

            _etile = {}

            def pqt(name):
                return pp.tile([128, 512], f32, tag="pq", bufs=3, name=name)

            def pst(name):
                return pp.tile([128, 512], f32, tag="ps", bufs=2, name=name)

            def warm1():
                pw = pst("pw")
                nc.tensor.matmul(pw[:], wrm[:, 0:128], wrm[:],
                                 start=True, stop=True)

            # ------- PE warmup: spin the HAM up while inputs stream -------
            for wi in range(15):
                warm1()

            # ---------------- helpers ----------------
            def proj_kv_q0_sc(sc):
                """kv + q0 over x-tiles for one sc half; ACT drains after.

                On the sc0 (DMA-paced) pass, a warmup matmul is inserted at
                each x-chunk boundary to keep the PE p-state hot while the
                next chunk lands.
                """
                ak = pqt("pak")
                aq = pqt("paq")
                for t in range(DT):
                    dp = 128 if t < DT - 1 else DIM - 128 * (DT - 1) + 1
                    st, sp = (t == 0), (t == DT - 1)
                    nc.tensor.matmul(
                        ak[:], wkv_sb[:dp, 128 * t:128 * (t + 1)],
                        x_sb[t][:dp, 512 * sc:512 * (sc + 1)],
                        start=st, stop=sp)
                    nc.tensor.matmul(
                        aq[:], wq_sb[:dp, 128 * t:128 * (t + 1)],
                        x_sb[t][:dp, 512 * sc:512 * (sc + 1)],
                        start=st, stop=sp)
                nc.scalar.activation(kv_sb[:, 512 * sc:512 * (sc + 1)],
                                     ak[:], AF.Copy)
                nc.scalar.activation(qT[0][:, 512 * sc:512 * (sc + 1)],
                                     aq[:], AF.Copy)

            def v_transpose():
                """vT via PE transpose (identity rhs); append ones column."""
                for j in range(NJ):
                    pvt = pp.tile([128, 64], bf16, tag="pv", bufs=1,
                                  name="pv")
                    pv = pvt[:]
                    nc.tensor.matmul(
                        pv, kv_sb[64:128, 128 * j:128 * (j + 1)],
                        id_sb[64:128, 0:64],
                        start=True, stop=True, is_transpose=True)
                    nc.scalar.activation(v_sb[j][:, 0:64], pv, AF.Copy)
                    nc.vector.tensor_copy(v_sb[j][:, 64:65], ones0[:, 0:1])

            def proj_group(dst, g):
                """2x23 matmuls (ap=512) into ping-pong psums, ACT drains."""
                for sc in range(2):
                    pq = pqt("pq")
                    for t in range(DT):
                        dp = 128 if t < DT - 1 else DIM - 128 * (DT - 1) + 1
                        nc.tensor.matmul(
                            pq[:], wq_sb[:dp, (g * DT + t) * 128:
                                         (g * DT + t + 1) * 128],
                            x_sb[t][:dp, 512 * sc:512 * (sc + 1)],
                            start=(t == 0), stop=(t == DT - 1))
                    nc.scalar.activation(dst[:, 512 * sc:512 * (sc + 1)],
                                         pq[:], AF.Copy)

            def rope(dst, src, cos, sin, npart):
                """dst = src*cos + perm(src)*sin via PE perm + DVE mults."""
                for half in range(2):
                    cs = slice(512 * half, 512 * (half + 1))
                    psw = pqt("psw")
                    nc.tensor.matmul(psw[:npart, :], pm_sb[:npart, :npart],
                                     src[:npart, cs], start=True, stop=True)
                    tmp = rp.tile([128, 512], bf16, tag="tmp")
                    qc = rp.tile([128, 512], bf16, tag="qc")
                    nc.vector.tensor_tensor(tmp[:npart], psw[:npart, :],
                                            sin[:npart, cs], op=OP.mult)
                    nc.vector.tensor_tensor(qc[:npart], src[:npart, cs],
                                            cos[:npart, cs], op=OP.mult)
                    nc.vector.tensor_tensor(dst[:npart, cs], qc[:npart],
                                            tmp[:npart], op=OP.add)

            def et_tile(h, J):
                """-> (tile, col offset) of head h's 256-span at j-block J."""
                gp, par = h // 4, h % 2
                off = ((h // 2) % 2) * 256
                if J in (2, 3):
                    return eT23[(gp, par, J)], off
                return _etile[(gp, par, J)], off

            def scores_pair(gp, par, half, jp):
                """scores -> exp -> mask for 2 heads (groups 2gp, 2gp+1) of
                one row parity, 2 j-blocks.  One 512-wide matmul per
                j-block spans both heads' 256-col i-windows."""
                r0 = 64 * par
                for J in range(4 * half + 2 * jp, 4 * half + 2 * jp + 2):
                    ilo = 128 * J
                    nc2 = min(256, S - ilo)          # J=7 span is 128
                    ps = pst("ps")
                    nc.tensor.matmul(
                        ps[:, 0:2 * nc2],
                        kr_sb[r0:r0 + 64, 128 * J:128 * (J + 1)],
                        qRp[gp][r0:r0 + 64, 0:2, ilo:ilo + nc2],
                        start=True, stop=True)
                    eu = eU.tile([128, 512], bf16, tag="eu")
                    nc.scalar.activation(eu[:, :2 * nc2], ps[:, :2 * nc2],
                                         AF.Exp)
                    if J in (2, 3):
                        et = eT23[(gp, par, J)]
                    else:
                        et = eP.tile([128, 512], bf16, tag="et",
                                     name=f"et{gp}{par}_{J}")
                        _etile[(gp, par, J)] = et
                    if nc2 == 256:
                        nc.vector.tensor_tensor(et[:], eu[:],
                                                mk_sb[:], op=OP.mult)
                    else:                            # J=7: split per head
                        nc.vector.tensor_tensor(
                            et[:, 0:128], eu[:, 0:128], mk_sb[:, 0:128],
                            op=OP.mult)
                        nc.vector.tensor_tensor(
                            et[:, 256:384], eu[:, 128:256], mk_sb[:, 0:128],
                            op=OP.mult)

            def c_head_attn(h, half):
                """attnT fused 256-wide per j-block into PSUM (lazy zero)."""
                p, r0 = h // 2, 64 * (h % 2)
                dr = 32 * (h % 4)
                pb = pp.tile([65, 512], f32, tag="pb", bufs=2, name="pb")
                I0 = 4 * half
                first = True
                for J in range(max(0, I0 - 1), I0 + 4):
                    tl, off = et_tile(h, J)
                    ec0, el = off, 256
                    lo = 128 * (J - I0)
                    if J == I0 - 1:          # right half only (i-block I0)
                        ec0, el, lo = off + 128, 128, 0
                    elif J == I0 + 3:        # left half only (i-block I0+3)
                        el = 128
                    nc.tensor.matmul(
                        pb[:, lo:lo + el], v_sb[J][:, 0:65],
                        tl[:, ec0:ec0 + el],
                        start=first, stop=(J == I0 + 3),
                        skip_group_check=True)
                    first = False
                nc.vector.tensor_scalar_add(
                    dn[2 * half + h // 4][dr:dr + 1, :], pb[64:65, :],
                    es_sb[dr:dr + 1, (h // 4):(h // 4) + 1])
                nc.scalar.activation(atr[p][r0:r0 + 64,
                                            512 * half:512 * (half + 1)],
                                     pb[0:64, :], AF.Copy)

            def c_epilogue(half):
                with nc.allow_low_precision(reason="bf16 attn scale"):
                    for hg in range(2):
                        x = 2 * half + hg
                        nc.vector.reciprocal_approx_fast(rscr[:], dn[x][:])
                        nc.vector.tensor_copy(rdnb[x][:], rscr[:])
                for p in range(4):
                    prt = pst("prt")
                    nc.tensor.matmul(
                        prt[:], sel_sb[:, 128 * (p % 2):128 * (p % 2 + 1)],
                        rdnb[2 * half + p // 2][:], start=True, stop=True)
                    pc = eU.tile([128, 512], bf16, tag="prtc")
                    nc.scalar.activation(pc[:], prt[:], AF.Copy)
                    cs = slice(512 * half, 512 * (half + 1))
                    nc.vector.tensor_tensor(atf[p][:, cs], atr[p][:, cs],
                                            pc[:], op=OP.mult)

            def d_block(it):
                obt = eU.tile([128, DIM], bf16, tag="ob", bufs=3, name="obt")
                eng = nc.sync if it % 2 == 0 else nc.scalar
                alt = nc.scalar if it % 2 == 0 else nc.sync
                for dd in range(6):
                    po = pqt("po")
                    for et in range(4):
                        nc.tensor.matmul(
                            po[:, 0:DDC],
                            atf[et][:, 128 * it:128 * (it + 1)],
                            wo_sb[:, DIM * et + DDC * dd:
                                  DIM * et + DDC * (dd + 1)],
                            start=(et == 0), stop=(et == 3))
                    if dd % 2 == 0:
                        nc.scalar.activation(
                            obt[:, DDC * dd:DDC * (dd + 1)], po[:, 0:DDC],
                            AF.Copy)
                    else:
                        nc.vector.tensor_copy(
                            obt[:, DDC * dd:DDC * (dd + 1)], po[:, 0:DDC])
                    if it >= 6 and dd == 2:
                        eng.dma_start(
                            out_d[128 * it:128 * (it + 1), 0:3 * DDC],
                            obt[:, 0:3 * DDC])
                if it >= 6:
                    alt.dma_start(
                        out_d[128 * it:128 * (it + 1), 3 * DDC:DIM],
                        obt[:, 3 * DDC:DIM])
                else:
                    eng.dma_start(out_d[128 * it:128 * (it + 1), :], obt[:])

            # ---------------- Phase A + B + C-L (interleaved) -------------
            proj_kv_q0_sc(0)
            nc.scalar.dma_start(wq_sb[:, GQ:2 * GQ], wqg[:, GQ:2 * GQ])
            nc.sync.dma_start(wq_sb[:, 3 * GQ:4 * GQ], wqg[:, 3 * GQ:4 * GQ])
            proj_kv_q0_sc(1)
            nc.sync.dma_start(wq_sb[:, 2 * GQ:3 * GQ], wqg[:, 2 * GQ:3 * GQ])
            nc.sync.dma_start(pm_sb[:], perm[:])
            nc.sync.dma_start(id_sb[:], idm[:])
            nc.sync.dma_start(ck_sb[:], cosk[:])
            nc.sync.dma_start(sk_sb[:], sinkt[:])
            proj_group(qT[1], 1)
            nc.scalar.dma_start(cq_sb[:], cosq[:])
            nc.scalar.dma_start(sq_sb[:], sinq[:])
            nc.scalar.dma_start(mk_sb[:], mask01[:])
            nc.scalar.dma_start(sel_sb[:], sel2[:])
            nc.scalar.dma_start(es_sb[:], es2[:])
            nc.sync.dma_start(wo_sb[:, 0:2 * DIM], wog[:, 0:2 * DIM])
            v_transpose()
            rope(kr_sb, kv_sb, ck_sb, sk_sb, 64)
            nc.sync.dma_start(kr_sb[64:128, :], kr_sb[0:64, :])
            nc.scalar.dma_start(wo_sb[:, 2 * DIM:4 * DIM],
                                wog[:, 2 * DIM:4 * DIM])
            rope(qR[0], qT[0], cq_sb, sq_sb, 128)
            rope(qR[1], qT[1], cq_sb, sq_sb, 128)
            scores_pair(0, 0, 0, 0)
            scores_pair(0, 0, 0, 1)
            scores_pair(0, 1, 0, 0)
            scores_pair(0, 1, 0, 1)
            proj_group(qT[2], 2)
            rope(qR[2], qT[2], cq_sb, sq_sb, 128)
            c_head_attn(0, 0)
            c_head_attn(1, 0)
            proj_group(qT[3], 3)
            rope(qR[3], qT[3], cq_sb, sq_sb, 128)
            c_head_attn(2, 0)
            scores_pair(1, 0, 0, 0)
            scores_pair(1, 0, 0, 1)
            c_head_attn(3, 0)
            scores_pair(1, 1, 0, 0)
            scores_pair(1, 1, 0, 1)
            c_head_attn(4, 0)
            c_head_attn(5, 0)
            c_head_attn(6, 0)
            c_head_attn(7, 0)

            # ---------------- C-R interleaved with L-epi and D-left -------
            scores_pair(0, 0, 1, 0)
            scores_pair(0, 0, 1, 1)
            scores_pair(0, 1, 1, 0)
            scores_pair(0, 1, 1, 1)
            c_head_attn(0, 1)
            c_epilogue(0)
            c_head_attn(1, 1)
            d_block(0)
            c_head_attn(2, 1)
            c_head_attn(3, 1)
            scores_pair(1, 0, 1, 0)
            scores_pair(1, 0, 1, 1)
            d_block(1)
            c_head_attn(4, 1)
            scores_pair(1, 1, 1, 0)
            scores_pair(1, 1, 1, 1)
            c_head_attn(5, 1)
            d_block(2)
            c_head_attn(6, 1)
            c_head_attn(7, 1)
            c_epilogue(1)
            d_block(3)
            for it in range(4, NJ):
                d_block(it)

    nc.compile()
    return nc


def _host_prep(x, wq_w, wq_b, wk_w, wk_b, wv_w, wv_b, wo_w, wo_b, sinks):
    """Build per-core input maps (host-side sharding + bf16 layout prep)."""
    import ml_dtypes
    bf = ml_dtypes.bfloat16
    f = np.float32
    xT = np.ascontiguousarray(x.reshape(S, DIM).T).astype(f)   # [2880, 1024]
    xt = np.zeros((128, DT * S), f)
    for t in range(DT):
        dp = min(128, DIM - 128 * t)
        xt[:dp, S * t:S * (t + 1)] = xT[128 * t:128 * t + dp]
    xt[64, S * (DT - 1):] = 1.0                                # bias row
    xt = xt.astype(bf)

    half = HD // 2
    inv_freq = 1.0 / (THETA ** (np.arange(half, dtype=np.float64) * 2.0 / HD))
    ang = np.arange(S, dtype=np.float64)[:, None] * inv_freq   # [S, 32]
    cos_t = np.cos(ang).T.astype(f)                            # [32, S]
    sin_t = np.sin(ang).T.astype(f)
    cos64 = np.concatenate([cos_t, cos_t], 0)                  # [64, S]
    sin64 = np.concatenate([sin_t, sin_t], 0)                  # sign in perm
    scale = np.float32(HD ** -0.5)
    cosq = (np.concatenate([cos64, cos64], 0) * scale).astype(bf)
    sinq = (np.concatenate([sin64, sin64], 0) * scale).astype(bf)
    cosk = cos64.astype(bf)
    sinkt = sin64.astype(bf)

    # signed rotate-half permutation, as matmul lhsT: perm[src, a] = sign
    # out[a] = -in[a+32] for a%64<32 else +in[a-32]
    perm = np.zeros((128, 128), f)
    for a in range(128):
        if (a // 32) % 2 == 0:
            perm[a + 32, a] = -1.0
        else:
            perm[a - 32, a] = 1.0
    perm = perm.astype(bf)

    idm = np.zeros((128, 64), f)
    for i in range(64):
        idm[64 + i, i] = 1.0
    idm = idm.astype(bf)

    jj = np.arange(128)[:, None]
    ii = np.arange(512)[None, :]
    ib = ii % 256
    allow_l = (jj <= ib) & (ib < 128)
    allow_r = (ib >= 128) & (jj > ib - 128)
    mask01 = np.where(allow_l | allow_r, 1.0, 0.0).astype(bf)  # [128, 512]

    sel2 = np.zeros((128, 256), f)
    for s in range(2):                       # selA: rows 0,32; selB: 64,96
        sel2[64 * s, 128 * s:128 * s + 64] = 1.0
        sel2[64 * s + 32, 128 * s + 64:128 * (s + 1)] = 1.0
    sel2 = sel2.astype(bf)

    def tileT(w, b):
        # w [E, DIM] (+ bias b [E]) -> [128, DT*E] tiled transpose, bias@row64
        E = w.shape[0]
        o = np.zeros((128, DT * E), f)
        for t in range(DT):
            dp = min(128, DIM - 128 * t)
            o[:dp, E * t:E * (t + 1)] = w[:, 128 * t:128 * t + dp].T
        o[64, E * (DT - 1):] = b
        return o

    def esink_layout(s8):
        out = np.zeros((128, 2), f)
        for h in range(HL):
            out[32 * (h % 4), h // 4] = np.exp(np.float64(s8[h]))
        return out

    in_maps = []
    for c in range(NC):
        wq_c = wq_w[EL * c:EL * (c + 1)]                  # [512, 2880]
        wqb_c = wq_b[EL * c:EL * (c + 1)]
        wqg = np.zeros((128, 4 * DT * 128), f)
        for g in range(4):
            wqg[:, g * DT * 128:(g + 1) * DT * 128] = tileT(
                wq_c[128 * g:128 * (g + 1)], wqb_c[128 * g:128 * (g + 1)])
        wkv_c = np.concatenate([wk_w[HD * c:HD * (c + 1)],
                                wv_w[HD * c:HD * (c + 1)]], 0)
        wkvb_c = np.concatenate([wk_b[HD * c:HD * (c + 1)],
                                 wv_b[HD * c:HD * (c + 1)]])
        wo_c = np.ascontiguousarray(wo_w[:, EL * c:EL * (c + 1)].T)
        wog = np.zeros((128, 4 * DIM), f)
        for et in range(4):
            wog[:, DIM * et:DIM * (et + 1)] = wo_c[128 * et:128 * (et + 1)]
        in_maps.append({
            "xt": xt,
            "wqg": wqg.astype(bf),
            "wkvg": tileT(wkv_c, wkvb_c).astype(bf),
            "wog": wog.astype(bf),
            "cosq": cosq, "sinq": sinq, "cosk": cosk, "sinkt": sinkt,
            "perm": perm, "idm": idm, "mask01": mask01, "sel2": sel2,
            "es2": esink_layout(sinks[HL * c:HL * (c + 1)]),
        })
    return in_maps


def run_on_hw(inputs, trace=False, **kw):
    from concourse import bass_utils
    if "nc" not in _cache:
        _cache["nc"] = _build_module()
    in_maps = _host_prep(**inputs)
    res = bass_utils.run_bass_kernel_spmd(
        _cache["nc"], in_maps, core_ids=list(range(NC)), trace=trace, **kw)
    out = np.zeros((S, DIM), np.float64)
    for c in range(NC):
        out += res.results[c]["out"].astype(np.float64)
    out = (out + inputs["wo_b"].astype(np.float64)).astype(np.float32)
    return out.reshape(B, S, DIM), res


def kernel(**inputs) -> np.ndarray:
    out, _ = run_on_hw(inputs, trace=False)
    return out
